# revision 30
# baseline (speedup 1.0000x reference)
"""Soft-MoE discrete-action transition network — Trainium2 Bass kernel.

Problem shapes (hardcoded):
  obs [B=64, M=256, D=256] f32, action [B=64] i64,
  phi [D, E=4, S=64] f32, w1 [E, D, H=512] f32, b1 [E, H] f32 (zeros),
  w2 [E, H, A*D=4608] f32, b2 [E, A*D] f32 (zeros).  Output [B, M, D] f32.

Strategy:
  * Host gathers the action-selected slice of w2/b2 per batch element
    (w2sel[b] = w2[:, :, a_b*D:(a_b+1)*D]) — the one-hot contraction at the
    end of the reference selects exactly one D-wide block per batch, so
    doing the selection first cuts the dominant matmuls by A=18x.
  * Data-parallel over batch: 8 batch elements per NeuronCore, params
    replicated, no collectives.
  * Per batch, on device (P=128 partition chunks):
      logits  [m,es] = obsT.T @ phi      (lhsT=obsT[d,m], rhs=phi[d,es])
      logitsT [es,m] = phi.T  @ obsT     (lhsT=phi, rhs=obsT — same operands)
      exp both (ScalarE, accum_out gives the softmax denominators for free)
      slotsT  [d,es] = obs.T @ exp_l     (unnormalized dispatch)
      pre_h   [h,es] = w1_e.T @ slotsT   per expert; ReLU (dispatch softmax
                        normalizer folded past ReLU — valid since b1 == 0)
      y       [es,d] = h_e.T @ w2sel_e; scale rows by 1/colsum (dispatch)
      out     [m,d]  = exp_lT.T @ y; scale rows by 1/rowsum (combine)
"""

import os
import sys
import time

import numpy as np

for _p in ("/opt/trn_rl_repo",):
    if os.path.isdir(_p) and _p not in sys.path:
        sys.path.append(_p)

import concourse.bass as bass
import concourse.mybir as mybir
import concourse.tile as tile
from concourse import bacc
from concourse.bass import ds, ts

B, M, D, A = 64, 256, 256, 18
E, S, H = 4, 64, 512
ES = E * S
N_CORES = 8
BPC = B // N_CORES  # batches per core
P = 128
F32 = mybir.dt.float32

AF = mybir.ActivationFunctionType

# Matmul operand dtypes. float32r reinterprets fp32 operands for the PE's
# fast path (1 cycle/row at n>=256 vs 4 for plain fp32). dt_y controls the
# h @ w2sel stage (w2sel dominates DMA traffic; fp16 halves it).
MM_DT = getattr(mybir.dt, os.environ.get("MOE_MM_DT", "float16"))
Y_DT = getattr(mybir.dt, os.environ.get("MOE_Y_DT", "float16"))


def build_nc(mm_dt=F32, y_dt=None, has_b2=False, *, w1_late=True, ysc="alt",
             io_bufs=3, mid_bufs=3, o_dt=F32, lg_bufs=1, share_lg=False,
             y_bufs=2, ou_bufs=1, split_start=False, PIPELINED_EMIT=False,
             dedup=True, merge_oo=False, w2_one=False):
    """Build the per-core Bass program (one NeuronCore, BPC batches)."""
    if y_dt is None:
        y_dt = mm_dt
    nc = bacc.Bacc("TRN2", target_bir_lowering=False, debug=False)

    # All tensors are pre-rearranged on the host into the exact SBUF layouts,
    # so every DMA is a contiguous [128, N] copy.
    if merge_oo:
        oo_d = nc.dram_tensor(
            "oo", [BPC, P, 2 * D + 2 * M], mm_dt, kind="ExternalInput"
        ).ap()
    else:
        obs_d = nc.dram_tensor(
            "obs", [BPC, P, 2 * D], mm_dt, kind="ExternalInput"
        ).ap()
        obsT_d = nc.dram_tensor(
            "obsT", [BPC, P, 2 * M], mm_dt, kind="ExternalInput"
        ).ap()
    phi_d = nc.dram_tensor("phi", [P, 2 * ES], mm_dt, kind="ExternalInput").ap()
    w1_d = nc.dram_tensor("w1", [P, 2 * E * H], mm_dt, kind="ExternalInput").ap()
    w2_d = nc.dram_tensor(
        "w2sel", [BPC, P, E * 4 * D], y_dt, kind="ExternalInput"
    ).ap()
    if has_b2:
        b2_d = nc.dram_tensor(
            "b2sel", [BPC, 1, E * D], y_dt, kind="ExternalInput"
        ).ap()
    if dedup:
        flag_d = nc.dram_tensor(
            "w2flag", [1, BPC], mybir.dt.int32, kind="ExternalInput"
        ).ap()
    out_d = nc.dram_tensor("out", [BPC, P, 2 * D], o_dt, kind="ExternalOutput").ap()

    with tile.TileContext(nc) as tc:
        with (
            tc.tile_pool(name="const", bufs=1) as const,
            tc.tile_pool(name="io", bufs=io_bufs) as io,
            tc.tile_pool(name="mid", bufs=mid_bufs) as mid,
            tc.tile_pool(name="psum", bufs=1, space="PSUM") as psp,
        ):
            phi_sb = const.tile([P, 2, ES], mm_dt)
            if split_start:
                phi_v = phi_d.rearrange("p (c s) -> p c s", c=2)
                for dc in range(2):
                    nc.sync.dma_start(out=phi_sb[:, dc, :], in_=phi_v[:, dc, :])
            else:
                nc.sync.dma_start(out=phi_sb, in_=phi_d)
            w1_sb = const.tile([P, 2, E, H], mm_dt)
            if not w1_late:
                nc.sync.dma_start(out=w1_sb, in_=w1_d)
            if dedup:
                # batches are host-sorted by action; w2sel lives in TWO
                # alternating persistent tiles (parity ib%2) and is re-loaded
                # only when the action differs from two batches back
                # (runtime-conditional DMA, flags from the w2flag input).
                w2_fix0 = const.tile([P, E, 4, D], y_dt)
                w2_fix1 = const.tile([P, E, 4, D], y_dt)
                w2_fix = [w2_fix0, w2_fix1]
                flags_sb = const.tile([1, BPC], mybir.dt.int32)
                nc.sync.dma_start(out=flags_sb, in_=flag_d)

            def stage1(ib):
                if merge_oo:
                    # obs and obsT ride one DMA; host stores them adjacently
                    oo_sb = io.tile([P, 4, D], mm_dt, tag="oo")
                    nc.sync.dma_start(
                        out=oo_sb, in_=oo_d[ib].rearrange("p (c d) -> p c d", c=4)
                    )
                    obsT_sb = oo_sb[:, 2:4, :]
                    obs_sb = oo_sb[:, 0:2, :]
                else:
                    obsT_sb = io.tile([P, 2, M], mm_dt, tag="obsT")
                    nc.sync.dma_start(out=obsT_sb, in_=obsT_d[ib])
                    obs_sb = io.tile([P, 2, D], mm_dt, tag="obs")
                    nc.sync.dma_start(out=obs_sb, in_=obs_d[ib])
                if ib == 0 and w1_late:
                    # logits only need phi+obsT, so deferring the w1 const
                    # load lets PE start ~3us earlier.
                    nc.sync.dma_start(out=w1_sb, in_=w1_d)
                w2_src = w2_d[ib].rearrange("p (e k) -> p e k", e=E)
                if dedup:
                    w2_sb = w2_fix[ib % 2]
                    if ib < 2:
                        for e in range(E):
                            nc.sync.dma_start(out=w2_sb[:, e], in_=w2_src[:, e])
                    else:
                        cv = nc.sync.value_load(
                            flags_sb[0:1, ib : ib + 1], min_val=0, max_val=1
                        )
                        for e in range(E):
                            nc.sync.dma_start(
                                out=w2_sb[:, e], in_=w2_src[:, e],
                                cond=cv, cond_hint=False,
                            )
                else:
                    w2_sb = io.tile([P, E, 4, D], y_dt, tag="w2")
                    if w2_one:
                        nc.sync.dma_start(out=w2_sb, in_=w2_src)
                    else:
                        for e in range(E):
                            nc.sync.dma_start(out=w2_sb[:, e], in_=w2_src[:, e])
                if has_b2:
                    # broadcast b2sel[e] across the 64 slot partitions of
                    # each expert: two 0-stride partition DMAs (pg = e % 2)
                    b2_bc = io.tile([P, 2, D], mm_dt, tag="b2")
                    for pg in range(2):
                        srcap = bass.AP(
                            tensor=b2_d.tensor,
                            offset=ib * E * D + pg * D,
                            ap=[[0, S], [2 * D, 2], [1, D]],
                        )
                        nc.sync.dma_start(
                            out=b2_bc[pg * S : (pg + 1) * S, :, :], in_=srcap
                        )

                # logits [m, es] (2 m-chunks), contracting d (2 chunks)
                lg_ps = psp.tile([P, 2, ES], F32, tag="lg", bufs=lg_bufs)
                for mc in range(2):
                    for dc in range(2):
                        nc.tensor.matmul(
                            lg_ps[:, mc, :],
                            obsT_sb[:, dc, ts(mc, P)],
                            phi_sb[:, dc, :],
                            start=(dc == 0),
                            stop=(dc == 1),
                        )
                exp_l = mid.tile([P, 2, ES], mm_dt, tag="expl")
                rsum = mid.tile([P, 2], F32, tag="rsum")
                for mc in range(2):
                    nc.scalar.activation(
                        exp_l[:, mc, :], lg_ps[:, mc, :], AF.Exp,
                        accum_out=rsum[:, mc : mc + 1],
                    )

                # logitsT [es, m] (2 es-chunks)
                lgT_ps = psp.tile([P, 2, M], F32, tag="lg" if share_lg else "lgT", bufs=lg_bufs if share_lg else 1)
                for ec in range(2):
                    for dc in range(2):
                        nc.tensor.matmul(
                            lgT_ps[:, ec, :],
                            phi_sb[:, dc, ts(ec, P)],
                            obsT_sb[:, dc, :],
                            start=(dc == 0),
                            stop=(dc == 1),
                        )
                exp_lT = mid.tile([P, 2, M], mm_dt, tag="explT")
                csum = mid.tile([P, 2], F32, tag="csum")
                for ec in range(2):
                    nc.scalar.activation(
                        exp_lT[:, ec, :], lgT_ps[:, ec, :], AF.Exp,
                        accum_out=csum[:, ec : ec + 1],
                    )

                recip_c = mid.tile([P, 2], F32, tag="rc")
                nc.vector.reciprocal(recip_c, rsum)
                recip_d = mid.tile([P, 2], F32, tag="rd")
                nc.vector.reciprocal(recip_d, csum)

                # slotsT [d, es] = obs.T @ exp_l (unnormalized dispatch)
                sl_ps = psp.tile([P, 2, ES], F32, tag="sl")
                for dc in range(2):
                    for mc in range(2):
                        nc.tensor.matmul(
                            sl_ps[:, dc, :],
                            obs_sb[:, mc, ts(dc, P)],
                            exp_l[:, mc, :],
                            start=(mc == 0),
                            stop=(mc == 1),
                        )
                slots_sb = mid.tile([P, 2, ES], mm_dt, tag="slots")
                nc.vector.tensor_copy(slots_sb, sl_ps)

                return (slots_sb, exp_lT, recip_c, recip_d, w2_sb,
                        b2_bc if has_b2 else None)

            def tail(ib, ctx):
                slots_sb, exp_lT, recip_c, recip_d, w2_sb, b2_bc = ctx
                # pre_h [h, (e,s)] per h-chunk; 4 h-chunks x 4 experts x 2 dc
                # (two half-tiles so ReLU on half 0 overlaps matmuls of half 1)
                h_sb = mid.tile([P, 4, ES], y_dt, tag="h")
                for half in range(2):
                    ph_ps = psp.tile([P, 2, ES], F32, tag="ph", bufs=2)
                    for hc2 in range(2):
                        hc = half * 2 + hc2
                        for e in range(E):
                            for dc in range(2):
                                nc.tensor.matmul(
                                    ph_ps[:, hc2, ds(e * S, S)],
                                    w1_sb[:, dc, e, ts(hc, P)],
                                    slots_sb[:, dc, ds(e * S, S)],
                                    start=(dc == 0),
                                    stop=(dc == 1),
                                )
                    nc.vector.tensor_scalar_max(
                        h_sb[:, half * 2 : half * 2 + 2, :], ph_ps, 0.0
                    )

                # y [es, d]: expert e -> es-chunk e//2, partition off (e%2)*64
                # Dispatch normalizer applied on the PSUM->SBUF copy; the four
                # copies alternate DVE/ACT to balance engine load.
                y_sb = mid.tile([P, 2, D], mm_dt, tag="ysb")
                for e in range(E):
                    ec, po = e // 2, (e % 2) * S
                    y_ps = psp.tile([S, D], F32, tag="y", bufs=y_bufs)
                    for hc in range(4):
                        nc.tensor.matmul(
                            y_ps,
                            h_sb[:, hc, ds(e * S, S)],
                            w2_sb[:, e, hc, :],
                            start=(hc == 0),
                            stop=(hc == 3),
                        )
                    use_dve = ysc == "dve" or (ysc == "alt" and e % 2 == 0)
                    if use_dve:
                        nc.vector.tensor_scalar_mul(
                            y_sb[po : po + S, ec, :], in0=y_ps,
                            scalar1=recip_d[po : po + S, ec : ec + 1],
                        )
                    else:
                        nc.scalar.activation(
                            y_sb[po : po + S, ec, :], y_ps, AF.Copy,
                            scale=recip_d[po : po + S, ec : ec + 1],
                        )
                    if has_b2:
                        nc.vector.tensor_add(
                            y_sb[po : po + S, ec, :],
                            y_sb[po : po + S, ec, :],
                            b2_bc[po : po + S, ec, :],
                        )

                # out [m, d] = exp_lT.T @ y, then combine normalization
                ou_ps = psp.tile([P, 2, D], F32, tag="ou", bufs=ou_bufs)
                for mc in range(2):
                    for ec in range(2):
                        nc.tensor.matmul(
                            ou_ps[:, mc, :],
                            exp_lT[:, ec, ts(mc, P)],
                            y_sb[:, ec, :],
                            start=(ec == 0),
                            stop=(ec == 1),
                        )
                out_sb = io.tile([P, 2, D], o_dt, tag="out")
                for mc in range(2):
                    nc.vector.tensor_scalar_mul(
                        out_sb[:, mc, :], in0=ou_ps[:, mc, :],
                        scalar1=recip_c[:, mc : mc + 1],
                    )
                nc.sync.dma_start(out=out_d[ib], in_=out_sb)

            if PIPELINED_EMIT:
                prev = None
                for ib in range(BPC):
                    ctx = stage1(ib)
                    if prev is not None:
                        tail(ib - 1, prev)
                    prev = ctx
                tail(BPC - 1, prev)
            else:
                for ib in range(BPC):
                    tail(ib, stage1(ib))

    nc.compile()
    return nc


class _Runner:
    """Compile once per process; re-execute via a cached jitted shard_map."""

    def __init__(self, mm_dt=F32, y_dt=None, has_b2=False):
        # The Tile PSUM slot allocator is heuristic and can spuriously fail
        # near capacity; retry a few times.
        last = None
        for _ in range(4):
            try:
                self.nc = build_nc(
                    mm_dt=mm_dt, y_dt=y_dt, has_b2=has_b2, dedup=DEDUP
                )
                break
            except ValueError as e:
                last = e
        else:
            raise last
        self.has_b2 = has_b2
        self._fn = None

    def _build_fn(self):
        import jax
        from jax.sharding import Mesh, PartitionSpec
        from jax.experimental.shard_map import shard_map
        from concourse import bass2jax
        from concourse.bass2jax import _bass_exec_p, partition_id_tensor

        bass2jax.install_neuronx_cc_hook()
        nc = self.nc
        partition_name = (
            nc.partition_id_tensor.name if nc.partition_id_tensor else None
        )
        in_names, out_names, out_avals, zero_outs = [], [], [], []
        for alloc in nc.m.functions[0].allocations:
            if not isinstance(alloc, mybir.MemoryLocationSet):
                continue
            name = alloc.memorylocations[0].name
            if alloc.kind == "ExternalInput":
                if name != partition_name:
                    in_names.append(name)
            elif alloc.kind == "ExternalOutput":
                shape = tuple(alloc.tensor_shape)
                dtype = mybir.dt.np(alloc.dtype)
                out_names.append(name)
                out_avals.append(jax.core.ShapedArray(shape, dtype))
                zero_outs.append(np.zeros(shape, dtype))
        n_params = len(in_names)
        all_in_names = list(in_names) + list(out_names)
        if partition_name is not None:
            all_in_names.append(partition_name)

        def _body(*args):
            operands = list(args)
            if partition_name is not None:
                operands.append(partition_id_tensor())
            outs = _bass_exec_p.bind(
                *operands,
                out_avals=tuple(out_avals),
                in_names=tuple(all_in_names),
                out_names=tuple(out_names),
                lowering_input_output_aliases=(),
                sim_require_finite=True,
                sim_require_nnan=True,
                nc=nc,
            )
            return tuple(outs)

        devices = jax.devices()[:N_CORES]
        assert len(devices) >= N_CORES, (
            f"need {N_CORES} NeuronCores, found {len(jax.devices())}"
        )
        mesh = Mesh(np.asarray(devices), ("core",))
        n_outs = len(out_names)
        sharded = jax.jit(
            shard_map(
                _body,
                mesh=mesh,
                in_specs=(PartitionSpec("core"),) * (n_params + n_outs),
                out_specs=(PartitionSpec("core"),) * n_outs,
                check_rep=False,
            ),
            donate_argnums=tuple(range(n_params, n_params + n_outs)),
            keep_unused=True,
        )
        self._in_names = in_names
        self._out_names = out_names
        self._out_avals = out_avals
        self._zero_outs = zero_outs
        self._fn = sharded

    def run(self, in_maps):
        """in_maps: list of N_CORES dicts name->np.ndarray. Returns per-core
        dict of outputs."""
        if self._fn is None:
            self._build_fn()
        concat_in = [
            np.concatenate([in_maps[c][nm] for c in range(N_CORES)], axis=0)
            for nm in self._in_names
        ]
        concat_zeros = [
            np.zeros((N_CORES * z.shape[0], *z.shape[1:]), z.dtype)
            for z in self._zero_outs
        ]
        out_arrs = self._fn(*concat_in, *concat_zeros)
        return [
            {
                nm: np.asarray(out_arrs[i]).reshape(
                    N_CORES, *self._out_avals[i].shape
                )[c]
                for i, nm in enumerate(self._out_names)
            }
            for c in range(N_CORES)
        ]


_runner_cache = {}


DEDUP = os.environ.get("MOE_DEDUP", "0") == "1"


def _prep_inputs(obs, action, phi, w1, b1, w2, b2):
    obs = np.ascontiguousarray(np.asarray(obs, dtype=np.float32))
    action = np.asarray(action).astype(np.int64)
    phi = np.asarray(phi, dtype=np.float32)
    w1 = np.ascontiguousarray(np.asarray(w1, dtype=np.float32))
    b1 = np.asarray(b1, dtype=np.float32)
    w2 = np.asarray(w2, dtype=np.float32)
    b2 = np.asarray(b2, dtype=np.float32)
    if np.any(b1):
        # The device kernel folds the dispatch-softmax normalizer past the
        # ReLU, which requires b1 == 0 (true for this problem's inputs).
        # Any other input falls back to an exact host computation.
        return None
    # Sort batches by action so equal-action batches are adjacent; the
    # kernel then skips re-loading w2sel when the action repeats. The
    # output rows are un-permuted at the end of kernel().
    if DEDUP:
        order = np.argsort(action, kind="stable")
    else:
        order = np.arange(B)
    obs = obs[order]
    action_s = action[order]
    flags = np.ones(B, np.int32)
    for b in range(B):
        if b % BPC >= 2 and action_s[b] == action_s[b - 2]:
            flags[b] = 0
    obsT = obs.transpose(0, 2, 1)
    # Pre-rearrange everything into the kernel's SBUF layouts (partition dim
    # first, contiguous free) so on-device DMAs are plain [128, N] copies.
    # obs [B,M,D] -> (b, p, mc, d): m = mc*128 + p
    obs_k = np.ascontiguousarray(
        obs.reshape(B, 2, P, D).transpose(0, 2, 1, 3)
    ).reshape(B, P, 2 * D)
    # obsT [B,D,M] -> (b, p, dc, m): d = dc*128 + p
    obsT_k = np.ascontiguousarray(
        obsT.reshape(B, 2, P, M).transpose(0, 2, 1, 3)
    ).reshape(B, P, 2 * M)
    # phi [D,ES] -> (p, dc, es)
    phi_k = np.ascontiguousarray(
        phi.reshape(2, P, ES).transpose(1, 0, 2)
    ).reshape(P, 2 * ES)
    # w1 [E,D,H] -> (p, dc, e, h)
    w1_k = np.ascontiguousarray(
        w1.reshape(E, 2, P, H).transpose(2, 1, 0, 3)
    ).reshape(P, 2 * E * H)
    # per-batch action-selected slices: w2sel [B,E,H,D] -> (b, p, e, hc, d)
    w2r = w2.reshape(E, H, A, D)
    w2sel = w2r[:, :, action_s, :].transpose(2, 0, 1, 3)  # [B,E,H,D]
    w2_k = np.ascontiguousarray(
        w2sel.reshape(B, E, 4, P, D).transpose(0, 3, 1, 2, 4)
    ).reshape(B, P, E * 4 * D)
    has_b2 = bool(np.any(b2))
    b2_k = None
    if has_b2:
        b2r = b2.reshape(E, A, D)
        b2_k = np.ascontiguousarray(
            b2r[:, action_s, :].transpose(1, 0, 2)
        ).reshape(B, 1, E * D)

    np_main = mybir.dt.np(MM_DT)
    np_y = mybir.dt.np(Y_DT)
    obs_k = obs_k.astype(np_main)
    obsT_k = obsT_k.astype(np_main)
    phi_k = phi_k.astype(np_main)
    w1_k = w1_k.astype(np_main)
    w2_k = w2_k.astype(np_y)
    if has_b2:
        b2_k = b2_k.astype(np_y)
    in_maps = []
    for c in range(N_CORES):
        sl = slice(c * BPC, (c + 1) * BPC)
        m = {
            "obs": obs_k[sl],
            "obsT": obsT_k[sl],
            "phi": phi_k,
            "w1": w1_k,
            "w2sel": w2_k[sl],
        }
        if DEDUP:
            m["w2flag"] = flags[sl].reshape(1, BPC)
        if has_b2:
            m["b2sel"] = b2_k[sl]
        in_maps.append(m)
    return in_maps, has_b2, order


def get_runner(has_b2, mm_dt=None, y_dt=None):
    if mm_dt is None:
        mm_dt = MM_DT
    if y_dt is None:
        y_dt = Y_DT
    key = (str(mm_dt), str(y_dt), has_b2)
    if key not in _runner_cache:
        _runner_cache[key] = _Runner(mm_dt=mm_dt, y_dt=y_dt, has_b2=has_b2)
    return _runner_cache[key]


def _numpy_reference(obs, action, phi, w1, b1, w2, b2):
    obs = np.asarray(obs, np.float64)
    logits = np.einsum("bmd,des->bmes", obs, np.asarray(phi, np.float64).reshape(D, E, S))
    lmax = logits.max(axis=1, keepdims=True)
    el = np.exp(logits - lmax)
    dispatch = el / el.sum(axis=1, keepdims=True)
    lf = logits.reshape(B, M, E * S)
    ec_ = np.exp(lf - lf.max(axis=-1, keepdims=True))
    combine = (ec_ / ec_.sum(axis=-1, keepdims=True)).reshape(B, M, E, S)
    slots = np.einsum("bmd,bmes->besd", obs, dispatch)
    h = np.maximum(
        np.einsum("besd,edh->besh", slots, np.asarray(w1, np.float64))
        + np.asarray(b1, np.float64)[None, :, None, :], 0
    )
    y = np.einsum("besh,ehk->besk", h, np.asarray(w2, np.float64)) + np.asarray(
        b2, np.float64
    )[None, :, None, :]
    out = np.einsum("bmes,besk->bmk", combine, y)
    out = out.reshape(B, M, A, D).transpose(0, 2, 1, 3)
    oh = np.eye(A)[np.asarray(action).astype(np.int64)]
    return np.einsum("bamd,ba->bmd", out, oh).astype(np.float32)


def kernel(obs, action, phi, w1, b1, w2, b2):
    prep = _prep_inputs(obs, action, phi, w1, b1, w2, b2)
    if prep is None:
        return _numpy_reference(obs, action, phi, w1, b1, w2, b2)
    in_maps, has_b2, order = prep
    runner = get_runner(has_b2)
    results = None
    last_err = None
    for attempt in range(3):
        try:
            results = runner.run(in_maps)
            break
        except Exception as e:  # transient device wedges recover on retry
            last_err = e
            time.sleep(2.0)
    if results is None:
        raise last_err
    out_k = np.concatenate([results[c]["out"] for c in range(N_CORES)], axis=0)
    # (b, p, mc, d) -> [B, M, D] with m = mc*128 + p; undo the action sort
    out_s = out_k.reshape(B, P, 2, D).transpose(0, 2, 1, 3).reshape(B, M, D)
    out = np.empty_like(out_s)
    out[order] = out_s
    return np.ascontiguousarray(out).astype(np.float32)


# revision 31
# speedup vs baseline: 2141.8722x; 2141.8722x over previous
"""Soft-MoE discrete-action transition network — Trainium2 Bass kernel.

Problem shapes (hardcoded):
  obs [B=64, M=256, D=256] f32, action [B=64] i64,
  phi [D, E=4, S=64] f32, w1 [E, D, H=512] f32, b1 [E, H] f32 (zeros),
  w2 [E, H, A*D=4608] f32, b2 [E, A*D] f32 (zeros).  Output [B, M, D] f32.

Strategy:
  * Host gathers the action-selected slice of w2/b2 per batch element
    (w2sel[b] = w2[:, :, a_b*D:(a_b+1)*D]) — the one-hot contraction at the
    end of the reference selects exactly one D-wide block per batch, so
    doing the selection first cuts the dominant matmuls by A=18x
    (~86 GFLOP -> ~13 GFLOP).
  * Data-parallel over batch: 8 batch elements per NeuronCore, params
    replicated, no collectives. All layout rearrangement happens on the
    host so every device DMA is a contiguous [128, N] copy.
  * Matmul operands are fp16 (fp32 would run the PE at 1/4 rate and double
    DMA); PSUM accumulation and softmax plumbing stay fp32. Measured
    end-to-end rel-l2 error ~3.3e-4.
  * Per batch, on device (P=128 partition chunks):
      logits  [m,es] = obsT.T @ phi      (lhsT=obsT[d,m], rhs=phi[d,es])
      logitsT [es,m] = phi.T  @ obsT     (lhsT=phi, rhs=obsT — same operands)
      exp both (ScalarE; accum_out yields both softmax denominators free)
      slotsT  [d,es] = obs.T @ exp_l     (unnormalized dispatch)
      pre_h   [h,es] = w1_e.T @ slotsT   per expert; ReLU (dispatch softmax
                        normalizer folded past ReLU — valid since b1 == 0;
                        nonzero b1 falls back to an exact host computation)
      y       [es,d] = h_e.T @ w2sel_e; scale rows by 1/colsum (dispatch);
                        nonzero b2 is added via a broadcast tile afterwards
      out     [m,d]  = exp_lT.T @ y; scale rows by 1/rowsum (combine)
  * Cost model (TimelineSim): ~55.8 us/core; engine busy: DMA 38us,
    PE 37us, DVE 29us, ACT 26us — a balanced "ridge" kernel.
"""

import os
import sys
import time

import numpy as np

for _p in ("/opt/trn_rl_repo",):
    if os.path.isdir(_p) and _p not in sys.path:
        sys.path.append(_p)

import concourse.bass as bass
import concourse.mybir as mybir
import concourse.tile as tile
from concourse import bacc
from concourse.bass import ds, ts

B, M, D, A = 64, 256, 256, 18
E, S, H = 4, 64, 512
ES = E * S
N_CORES = 8
BPC = B // N_CORES  # batches per core
P = 128
F32 = mybir.dt.float32

AF = mybir.ActivationFunctionType

# Matmul operand dtypes. float32r reinterprets fp32 operands for the PE's
# fast path (1 cycle/row at n>=256 vs 4 for plain fp32). dt_y controls the
# h @ w2sel stage (w2sel dominates DMA traffic; fp16 halves it).
MM_DT = getattr(mybir.dt, os.environ.get("MOE_MM_DT", "float16"))
Y_DT = getattr(mybir.dt, os.environ.get("MOE_Y_DT", "float16"))


def build_nc(mm_dt=F32, y_dt=None, has_b2=False, *, w1_late=True, ysc="alt",
             io_bufs=3, mid_bufs=3, o_dt=F32, lg_bufs=1, share_lg=False,
             y_bufs=2, ou_bufs=1, split_start=False, PIPELINED_EMIT=False,
             dedup=True, merge_oo=False, w2_one=False):
    """Build the per-core Bass program (one NeuronCore, BPC batches)."""
    if y_dt is None:
        y_dt = mm_dt
    nc = bacc.Bacc("TRN2", target_bir_lowering=False, debug=False)

    # All tensors are pre-rearranged on the host into the exact SBUF layouts,
    # so every DMA is a contiguous [128, N] copy.
    if merge_oo:
        oo_d = nc.dram_tensor(
            "oo", [BPC, P, 2 * D + 2 * M], mm_dt, kind="ExternalInput"
        ).ap()
    else:
        obs_d = nc.dram_tensor(
            "obs", [BPC, P, 2 * D], mm_dt, kind="ExternalInput"
        ).ap()
        obsT_d = nc.dram_tensor(
            "obsT", [BPC, P, 2 * M], mm_dt, kind="ExternalInput"
        ).ap()
    phi_d = nc.dram_tensor("phi", [P, 2 * ES], mm_dt, kind="ExternalInput").ap()
    w1_d = nc.dram_tensor("w1", [P, 2 * E * H], mm_dt, kind="ExternalInput").ap()
    w2_d = nc.dram_tensor(
        "w2sel", [BPC, P, E * 4 * D], y_dt, kind="ExternalInput"
    ).ap()
    if has_b2:
        b2_d = nc.dram_tensor(
            "b2sel", [BPC, 1, E * D], y_dt, kind="ExternalInput"
        ).ap()
    if dedup:
        flag_d = nc.dram_tensor(
            "w2flag", [1, BPC], mybir.dt.int32, kind="ExternalInput"
        ).ap()
    out_d = nc.dram_tensor("out", [BPC, P, 2 * D], o_dt, kind="ExternalOutput").ap()

    with tile.TileContext(nc) as tc:
        with (
            tc.tile_pool(name="const", bufs=1) as const,
            tc.tile_pool(name="io", bufs=io_bufs) as io,
            tc.tile_pool(name="mid", bufs=mid_bufs) as mid,
            tc.tile_pool(name="psum", bufs=1, space="PSUM") as psp,
        ):
            phi_sb = const.tile([P, 2, ES], mm_dt)
            if split_start:
                phi_v = phi_d.rearrange("p (c s) -> p c s", c=2)
                for dc in range(2):
                    nc.sync.dma_start(out=phi_sb[:, dc, :], in_=phi_v[:, dc, :])
            else:
                nc.sync.dma_start(out=phi_sb, in_=phi_d)
            w1_sb = const.tile([P, 2, E, H], mm_dt)
            if not w1_late:
                nc.sync.dma_start(out=w1_sb, in_=w1_d)
            if dedup:
                # batches are host-sorted by action; w2sel lives in TWO
                # alternating persistent tiles (parity ib%2) and is re-loaded
                # only when the action differs from two batches back
                # (runtime-conditional DMA, flags from the w2flag input).
                w2_fix0 = const.tile([P, E, 4, D], y_dt)
                w2_fix1 = const.tile([P, E, 4, D], y_dt)
                w2_fix = [w2_fix0, w2_fix1]
                flags_sb = const.tile([1, BPC], mybir.dt.int32)
                nc.sync.dma_start(out=flags_sb, in_=flag_d)

            def stage1(ib):
                if merge_oo:
                    # obs and obsT ride one DMA; host stores them adjacently
                    oo_sb = io.tile([P, 4, D], mm_dt, tag="oo")
                    nc.sync.dma_start(
                        out=oo_sb, in_=oo_d[ib].rearrange("p (c d) -> p c d", c=4)
                    )
                    obsT_sb = oo_sb[:, 2:4, :]
                    obs_sb = oo_sb[:, 0:2, :]
                else:
                    obsT_sb = io.tile([P, 2, M], mm_dt, tag="obsT")
                    nc.sync.dma_start(out=obsT_sb, in_=obsT_d[ib])
                    obs_sb = io.tile([P, 2, D], mm_dt, tag="obs")
                    nc.sync.dma_start(out=obs_sb, in_=obs_d[ib])
                if ib == 0 and w1_late:
                    # logits only need phi+obsT, so deferring the w1 const
                    # load lets PE start ~3us earlier.
                    nc.sync.dma_start(out=w1_sb, in_=w1_d)
                w2_src = w2_d[ib].rearrange("p (e k) -> p e k", e=E)
                if dedup:
                    w2_sb = w2_fix[ib % 2]
                    if ib < 2:
                        for e in range(E):
                            nc.sync.dma_start(out=w2_sb[:, e], in_=w2_src[:, e])
                    else:
                        cv = nc.sync.value_load(
                            flags_sb[0:1, ib : ib + 1], min_val=0, max_val=1
                        )
                        for e in range(E):
                            nc.sync.dma_start(
                                out=w2_sb[:, e], in_=w2_src[:, e],
                                cond=cv, cond_hint=False,
                            )
                else:
                    w2_sb = io.tile([P, E, 4, D], y_dt, tag="w2")
                    if w2_one:
                        nc.sync.dma_start(out=w2_sb, in_=w2_src)
                    else:
                        for e in range(E):
                            nc.sync.dma_start(out=w2_sb[:, e], in_=w2_src[:, e])
                if has_b2:
                    # broadcast b2sel[e] across the 64 slot partitions of
                    # each expert: two 0-stride partition DMAs (pg = e % 2)
                    b2_bc = io.tile([P, 2, D], mm_dt, tag="b2")
                    for pg in range(2):
                        srcap = bass.AP(
                            tensor=b2_d.tensor,
                            offset=ib * E * D + pg * D,
                            ap=[[0, S], [2 * D, 2], [1, D]],
                        )
                        nc.sync.dma_start(
                            out=b2_bc[pg * S : (pg + 1) * S, :, :], in_=srcap
                        )

                # logits [m, es] (2 m-chunks), contracting d (2 chunks)
                lg_ps = psp.tile([P, 2, ES], F32, tag="lg", bufs=lg_bufs)
                for mc in range(2):
                    for dc in range(2):
                        nc.tensor.matmul(
                            lg_ps[:, mc, :],
                            obsT_sb[:, dc, ts(mc, P)],
                            phi_sb[:, dc, :],
                            start=(dc == 0),
                            stop=(dc == 1),
                        )
                exp_l = mid.tile([P, 2, ES], mm_dt, tag="expl")
                rsum = mid.tile([P, 2], F32, tag="rsum")
                for mc in range(2):
                    nc.scalar.activation(
                        exp_l[:, mc, :], lg_ps[:, mc, :], AF.Exp,
                        accum_out=rsum[:, mc : mc + 1],
                    )

                # logitsT [es, m] (2 es-chunks)
                lgT_ps = psp.tile([P, 2, M], F32, tag="lg" if share_lg else "lgT", bufs=lg_bufs if share_lg else 1)
                for ec in range(2):
                    for dc in range(2):
                        nc.tensor.matmul(
                            lgT_ps[:, ec, :],
                            phi_sb[:, dc, ts(ec, P)],
                            obsT_sb[:, dc, :],
                            start=(dc == 0),
                            stop=(dc == 1),
                        )
                exp_lT = mid.tile([P, 2, M], mm_dt, tag="explT")
                csum = mid.tile([P, 2], F32, tag="csum")
                for ec in range(2):
                    nc.scalar.activation(
                        exp_lT[:, ec, :], lgT_ps[:, ec, :], AF.Exp,
                        accum_out=csum[:, ec : ec + 1],
                    )

                recip_c = mid.tile([P, 2], F32, tag="rc")
                nc.vector.reciprocal(recip_c, rsum)
                recip_d = mid.tile([P, 2], F32, tag="rd")
                nc.vector.reciprocal(recip_d, csum)

                # slotsT [d, es] = obs.T @ exp_l (unnormalized dispatch)
                sl_ps = psp.tile([P, 2, ES], F32, tag="sl")
                for dc in range(2):
                    for mc in range(2):
                        nc.tensor.matmul(
                            sl_ps[:, dc, :],
                            obs_sb[:, mc, ts(dc, P)],
                            exp_l[:, mc, :],
                            start=(mc == 0),
                            stop=(mc == 1),
                        )
                slots_sb = mid.tile([P, 2, ES], mm_dt, tag="slots")
                nc.vector.tensor_copy(slots_sb, sl_ps)

                return (slots_sb, exp_lT, recip_c, recip_d, w2_sb,
                        b2_bc if has_b2 else None)

            def tail(ib, ctx):
                slots_sb, exp_lT, recip_c, recip_d, w2_sb, b2_bc = ctx
                # pre_h [h, (e,s)] per h-chunk; 4 h-chunks x 4 experts x 2 dc
                # (two half-tiles so ReLU on half 0 overlaps matmuls of half 1)
                h_sb = mid.tile([P, 4, ES], y_dt, tag="h")
                for half in range(2):
                    ph_ps = psp.tile([P, 2, ES], F32, tag="ph", bufs=2)
                    for hc2 in range(2):
                        hc = half * 2 + hc2
                        for e in range(E):
                            for dc in range(2):
                                nc.tensor.matmul(
                                    ph_ps[:, hc2, ds(e * S, S)],
                                    w1_sb[:, dc, e, ts(hc, P)],
                                    slots_sb[:, dc, ds(e * S, S)],
                                    start=(dc == 0),
                                    stop=(dc == 1),
                                )
                    nc.vector.tensor_scalar_max(
                        h_sb[:, half * 2 : half * 2 + 2, :], ph_ps, 0.0
                    )

                # y [es, d]: expert e -> es-chunk e//2, partition off (e%2)*64
                # Dispatch normalizer applied on the PSUM->SBUF copy; the four
                # copies alternate DVE/ACT to balance engine load.
                y_sb = mid.tile([P, 2, D], mm_dt, tag="ysb")
                for e in range(E):
                    ec, po = e // 2, (e % 2) * S
                    y_ps = psp.tile([S, D], F32, tag="y", bufs=y_bufs)
                    for hc in range(4):
                        nc.tensor.matmul(
                            y_ps,
                            h_sb[:, hc, ds(e * S, S)],
                            w2_sb[:, e, hc, :],
                            start=(hc == 0),
                            stop=(hc == 3),
                        )
                    use_dve = ysc == "dve" or (ysc == "alt" and e % 2 == 0)
                    if use_dve:
                        nc.vector.tensor_scalar_mul(
                            y_sb[po : po + S, ec, :], in0=y_ps,
                            scalar1=recip_d[po : po + S, ec : ec + 1],
                        )
                    else:
                        nc.scalar.activation(
                            y_sb[po : po + S, ec, :], y_ps, AF.Copy,
                            scale=recip_d[po : po + S, ec : ec + 1],
                        )
                    if has_b2:
                        nc.vector.tensor_add(
                            y_sb[po : po + S, ec, :],
                            y_sb[po : po + S, ec, :],
                            b2_bc[po : po + S, ec, :],
                        )

                # out [m, d] = exp_lT.T @ y, then combine normalization
                ou_ps = psp.tile([P, 2, D], F32, tag="ou", bufs=ou_bufs)
                for mc in range(2):
                    for ec in range(2):
                        nc.tensor.matmul(
                            ou_ps[:, mc, :],
                            exp_lT[:, ec, ts(mc, P)],
                            y_sb[:, ec, :],
                            start=(ec == 0),
                            stop=(ec == 1),
                        )
                out_sb = io.tile([P, 2, D], o_dt, tag="out")
                for mc in range(2):
                    nc.vector.tensor_scalar_mul(
                        out_sb[:, mc, :], in0=ou_ps[:, mc, :],
                        scalar1=recip_c[:, mc : mc + 1],
                    )
                nc.sync.dma_start(out=out_d[ib], in_=out_sb)

            if PIPELINED_EMIT:
                prev = None
                for ib in range(BPC):
                    ctx = stage1(ib)
                    if prev is not None:
                        tail(ib - 1, prev)
                    prev = ctx
                tail(BPC - 1, prev)
            else:
                for ib in range(BPC):
                    tail(ib, stage1(ib))

    nc.compile()
    return nc


class _Runner:
    """Compile once per process; re-execute via a cached jitted shard_map."""

    def __init__(self, mm_dt=F32, y_dt=None, has_b2=False):
        # The Tile PSUM slot allocator is heuristic and can spuriously fail
        # near capacity; retry a few times.
        last = None
        for _ in range(4):
            try:
                self.nc = build_nc(
                    mm_dt=mm_dt, y_dt=y_dt, has_b2=has_b2, dedup=DEDUP
                )
                break
            except ValueError as e:
                last = e
        else:
            raise last
        self.has_b2 = has_b2
        self._fn = None

    def _build_fn(self):
        import jax
        from jax.sharding import Mesh, PartitionSpec
        from jax.experimental.shard_map import shard_map
        from concourse import bass2jax
        from concourse.bass2jax import _bass_exec_p, partition_id_tensor

        bass2jax.install_neuronx_cc_hook()
        nc = self.nc
        partition_name = (
            nc.partition_id_tensor.name if nc.partition_id_tensor else None
        )
        in_names, out_names, out_avals, zero_outs = [], [], [], []
        for alloc in nc.m.functions[0].allocations:
            if not isinstance(alloc, mybir.MemoryLocationSet):
                continue
            name = alloc.memorylocations[0].name
            if alloc.kind == "ExternalInput":
                if name != partition_name:
                    in_names.append(name)
            elif alloc.kind == "ExternalOutput":
                shape = tuple(alloc.tensor_shape)
                dtype = mybir.dt.np(alloc.dtype)
                out_names.append(name)
                out_avals.append(jax.core.ShapedArray(shape, dtype))
                zero_outs.append(np.zeros(shape, dtype))
        n_params = len(in_names)
        all_in_names = list(in_names) + list(out_names)
        if partition_name is not None:
            all_in_names.append(partition_name)

        def _body(*args):
            operands = list(args)
            if partition_name is not None:
                operands.append(partition_id_tensor())
            outs = _bass_exec_p.bind(
                *operands,
                out_avals=tuple(out_avals),
                in_names=tuple(all_in_names),
                out_names=tuple(out_names),
                lowering_input_output_aliases=(),
                sim_require_finite=True,
                sim_require_nnan=True,
                nc=nc,
            )
            return tuple(outs)

        devices = jax.devices()[:N_CORES]
        assert len(devices) >= N_CORES, (
            f"need {N_CORES} NeuronCores, found {len(jax.devices())}"
        )
        mesh = Mesh(np.asarray(devices), ("core",))
        n_outs = len(out_names)
        sharded = jax.jit(
            shard_map(
                _body,
                mesh=mesh,
                in_specs=(PartitionSpec("core"),) * (n_params + n_outs),
                out_specs=(PartitionSpec("core"),) * n_outs,
                check_rep=False,
            ),
            donate_argnums=tuple(range(n_params, n_params + n_outs)),
            keep_unused=True,
        )
        self._in_names = in_names
        self._out_names = out_names
        self._out_avals = out_avals
        self._zero_outs = zero_outs
        self._fn = sharded

    def run(self, in_maps):
        """in_maps: list of N_CORES dicts name->np.ndarray. Returns per-core
        dict of outputs."""
        if self._fn is None:
            self._build_fn()
        concat_in = [
            np.concatenate([in_maps[c][nm] for c in range(N_CORES)], axis=0)
            for nm in self._in_names
        ]
        concat_zeros = [
            np.zeros((N_CORES * z.shape[0], *z.shape[1:]), z.dtype)
            for z in self._zero_outs
        ]
        out_arrs = self._fn(*concat_in, *concat_zeros)
        return [
            {
                nm: np.asarray(out_arrs[i]).reshape(
                    N_CORES, *self._out_avals[i].shape
                )[c]
                for i, nm in enumerate(self._out_names)
            }
            for c in range(N_CORES)
        ]


_runner_cache = {}


DEDUP = os.environ.get("MOE_DEDUP", "0") == "1"


def _prep_inputs(obs, action, phi, w1, b1, w2, b2):
    obs = np.ascontiguousarray(np.asarray(obs, dtype=np.float32))
    action = np.asarray(action).astype(np.int64)
    phi = np.asarray(phi, dtype=np.float32)
    w1 = np.ascontiguousarray(np.asarray(w1, dtype=np.float32))
    b1 = np.asarray(b1, dtype=np.float32)
    w2 = np.asarray(w2, dtype=np.float32)
    b2 = np.asarray(b2, dtype=np.float32)
    if np.any(b1):
        # The device kernel folds the dispatch-softmax normalizer past the
        # ReLU, which requires b1 == 0 (true for this problem's inputs).
        # Any other input falls back to an exact host computation.
        return None
    # Sort batches by action so equal-action batches are adjacent; the
    # kernel then skips re-loading w2sel when the action repeats. The
    # output rows are un-permuted at the end of kernel().
    if DEDUP:
        order = np.argsort(action, kind="stable")
    else:
        order = np.arange(B)
    obs = obs[order]
    action_s = action[order]
    flags = np.ones(B, np.int32)
    for b in range(B):
        if b % BPC >= 2 and action_s[b] == action_s[b - 2]:
            flags[b] = 0
    obsT = obs.transpose(0, 2, 1)
    # Pre-rearrange everything into the kernel's SBUF layouts (partition dim
    # first, contiguous free) so on-device DMAs are plain [128, N] copies.
    # obs [B,M,D] -> (b, p, mc, d): m = mc*128 + p
    obs_k = np.ascontiguousarray(
        obs.reshape(B, 2, P, D).transpose(0, 2, 1, 3)
    ).reshape(B, P, 2 * D)
    # obsT [B,D,M] -> (b, p, dc, m): d = dc*128 + p
    obsT_k = np.ascontiguousarray(
        obsT.reshape(B, 2, P, M).transpose(0, 2, 1, 3)
    ).reshape(B, P, 2 * M)
    # phi [D,ES] -> (p, dc, es)
    phi_k = np.ascontiguousarray(
        phi.reshape(2, P, ES).transpose(1, 0, 2)
    ).reshape(P, 2 * ES)
    # w1 [E,D,H] -> (p, dc, e, h)
    w1_k = np.ascontiguousarray(
        w1.reshape(E, 2, P, H).transpose(2, 1, 0, 3)
    ).reshape(P, 2 * E * H)
    # per-batch action-selected slices: w2sel [B,E,H,D] -> (b, p, e, hc, d)
    w2r = w2.reshape(E, H, A, D)
    w2sel = w2r[:, :, action_s, :].transpose(2, 0, 1, 3)  # [B,E,H,D]
    w2_k = np.ascontiguousarray(
        w2sel.reshape(B, E, 4, P, D).transpose(0, 3, 1, 2, 4)
    ).reshape(B, P, E * 4 * D)
    has_b2 = bool(np.any(b2))
    b2_k = None
    if has_b2:
        b2r = b2.reshape(E, A, D)
        b2_k = np.ascontiguousarray(
            b2r[:, action_s, :].transpose(1, 0, 2)
        ).reshape(B, 1, E * D)

    np_main = mybir.dt.np(MM_DT)
    np_y = mybir.dt.np(Y_DT)
    obs_k = obs_k.astype(np_main)
    obsT_k = obsT_k.astype(np_main)
    phi_k = phi_k.astype(np_main)
    w1_k = w1_k.astype(np_main)
    w2_k = w2_k.astype(np_y)
    if has_b2:
        b2_k = b2_k.astype(np_y)
    in_maps = []
    for c in range(N_CORES):
        sl = slice(c * BPC, (c + 1) * BPC)
        m = {
            "obs": obs_k[sl],
            "obsT": obsT_k[sl],
            "phi": phi_k,
            "w1": w1_k,
            "w2sel": w2_k[sl],
        }
        if DEDUP:
            m["w2flag"] = flags[sl].reshape(1, BPC)
        if has_b2:
            m["b2sel"] = b2_k[sl]
        in_maps.append(m)
    return in_maps, has_b2, order


def get_runner(has_b2, mm_dt=None, y_dt=None):
    if mm_dt is None:
        mm_dt = MM_DT
    if y_dt is None:
        y_dt = Y_DT
    key = (str(mm_dt), str(y_dt), has_b2)
    if key not in _runner_cache:
        _runner_cache[key] = _Runner(mm_dt=mm_dt, y_dt=y_dt, has_b2=has_b2)
    return _runner_cache[key]


def _numpy_reference(obs, action, phi, w1, b1, w2, b2):
    obs = np.asarray(obs, np.float64)
    logits = np.einsum("bmd,des->bmes", obs, np.asarray(phi, np.float64).reshape(D, E, S))
    lmax = logits.max(axis=1, keepdims=True)
    el = np.exp(logits - lmax)
    dispatch = el / el.sum(axis=1, keepdims=True)
    lf = logits.reshape(B, M, E * S)
    ec_ = np.exp(lf - lf.max(axis=-1, keepdims=True))
    combine = (ec_ / ec_.sum(axis=-1, keepdims=True)).reshape(B, M, E, S)
    slots = np.einsum("bmd,bmes->besd", obs, dispatch)
    h = np.maximum(
        np.einsum("besd,edh->besh", slots, np.asarray(w1, np.float64))
        + np.asarray(b1, np.float64)[None, :, None, :], 0
    )
    y = np.einsum("besh,ehk->besk", h, np.asarray(w2, np.float64)) + np.asarray(
        b2, np.float64
    )[None, :, None, :]
    out = np.einsum("bmes,besk->bmk", combine, y)
    out = out.reshape(B, M, A, D).transpose(0, 2, 1, 3)
    oh = np.eye(A)[np.asarray(action).astype(np.int64)]
    return np.einsum("bamd,ba->bmd", out, oh).astype(np.float32)


def kernel(obs, action, phi, w1, b1, w2, b2):
    prep = _prep_inputs(obs, action, phi, w1, b1, w2, b2)
    if prep is None:
        return _numpy_reference(obs, action, phi, w1, b1, w2, b2)
    in_maps, has_b2, order = prep
    runner = get_runner(has_b2)
    results = None
    last_err = None
    for attempt in range(3):
        try:
            results = runner.run(in_maps)
            break
        except Exception as e:  # transient device wedges recover on retry
            last_err = e
            time.sleep(2.0)
    if results is None:
        raise last_err
    out_k = np.concatenate([results[c]["out"] for c in range(N_CORES)], axis=0)
    # (b, p, mc, d) -> [B, M, D] with m = mc*128 + p; undo the action sort
    out_s = out_k.reshape(B, P, 2, D).transpose(0, 2, 1, 3).reshape(B, M, D)
    out = np.empty_like(out_s)
    out[order] = out_s
    return np.ascontiguousarray(out).astype(np.float32)


# revision 32
# speedup vs baseline: 2144.9106x; 1.0014x over previous
"""Soft-MoE discrete-action transition network — Trainium2 Bass kernel.

Problem shapes (hardcoded):
  obs [B=64, M=256, D=256] f32, action [B=64] i64,
  phi [D, E=4, S=64] f32, w1 [E, D, H=512] f32, b1 [E, H] f32 (zeros),
  w2 [E, H, A*D=4608] f32, b2 [E, A*D] f32 (zeros).  Output [B, M, D] f32.

Strategy:
  * Host gathers the action-selected slice of w2/b2 per batch element
    (w2sel[b] = w2[:, :, a_b*D:(a_b+1)*D]) — the one-hot contraction at the
    end of the reference selects exactly one D-wide block per batch, so
    doing the selection first cuts the dominant matmuls by A=18x
    (~86 GFLOP -> ~13 GFLOP).
  * Data-parallel over batch: 8 batch elements per NeuronCore, params
    replicated, no collectives. All layout rearrangement happens on the
    host so every device DMA is a contiguous [128, N] copy.
  * Matmul operands are fp16 (fp32 would run the PE at 1/4 rate and double
    DMA); PSUM accumulation and softmax plumbing stay fp32. Measured
    end-to-end rel-l2 error ~3.3e-4.
  * Per batch, on device (P=128 partition chunks):
      logits  [m,es] = obsT.T @ phi      (lhsT=obsT[d,m], rhs=phi[d,es])
      logitsT [es,m] = phi.T  @ obsT     (lhsT=phi, rhs=obsT — same operands)
      exp both (ScalarE; accum_out yields both softmax denominators free)
      slotsT  [d,es] = obs.T @ exp_l     (unnormalized dispatch)
      pre_h   [h,es] = w1_e.T @ slotsT   per expert; ReLU (dispatch softmax
                        normalizer folded past ReLU — valid since b1 == 0;
                        nonzero b1 falls back to an exact host computation)
      y       [es,d] = h_e.T @ w2sel_e; scale rows by 1/colsum (dispatch);
                        nonzero b2 is added via a broadcast tile afterwards
      out     [m,d]  = exp_lT.T @ y; scale rows by 1/rowsum (combine)
  * Cost model (TimelineSim): ~55.8 us/core; engine busy: DMA 38us,
    PE 37us, DVE 29us, ACT 26us — a balanced "ridge" kernel.
"""

import os
import sys
import time

import numpy as np

for _p in ("/opt/trn_rl_repo",):
    if os.path.isdir(_p) and _p not in sys.path:
        sys.path.append(_p)

import concourse.bass as bass
import concourse.mybir as mybir
import concourse.tile as tile
from concourse import bacc
from concourse.bass import ds, ts

B, M, D, A = 64, 256, 256, 18
E, S, H = 4, 64, 512
ES = E * S
N_CORES = 8
BPC = B // N_CORES  # batches per core
P = 128
F32 = mybir.dt.float32

AF = mybir.ActivationFunctionType

# Matmul operand dtypes. float32r reinterprets fp32 operands for the PE's
# fast path (1 cycle/row at n>=256 vs 4 for plain fp32). dt_y controls the
# h @ w2sel stage (w2sel dominates DMA traffic; fp16 halves it).
MM_DT = getattr(mybir.dt, os.environ.get("MOE_MM_DT", "float16"))
Y_DT = getattr(mybir.dt, os.environ.get("MOE_Y_DT", "float16"))


def build_nc(mm_dt=F32, y_dt=None, has_b2=False, *, w1_late=True, ysc="dve",
             io_bufs=3, mid_bufs=3, o_dt=F32, lg_bufs=1, share_lg=False,
             y_bufs=2, ou_bufs=1, split_start=False, PIPELINED_EMIT=False,
             dedup=True, merge_oo=False, w2_one=False):
    """Build the per-core Bass program (one NeuronCore, BPC batches)."""
    if y_dt is None:
        y_dt = mm_dt
    nc = bacc.Bacc("TRN2", target_bir_lowering=False, debug=False)

    # All tensors are pre-rearranged on the host into the exact SBUF layouts,
    # so every DMA is a contiguous [128, N] copy.
    if merge_oo:
        oo_d = nc.dram_tensor(
            "oo", [BPC, P, 2 * D + 2 * M], mm_dt, kind="ExternalInput"
        ).ap()
    else:
        obs_d = nc.dram_tensor(
            "obs", [BPC, P, 2 * D], mm_dt, kind="ExternalInput"
        ).ap()
        obsT_d = nc.dram_tensor(
            "obsT", [BPC, P, 2 * M], mm_dt, kind="ExternalInput"
        ).ap()
    phi_d = nc.dram_tensor("phi", [P, 2 * ES], mm_dt, kind="ExternalInput").ap()
    w1_d = nc.dram_tensor("w1", [P, 2 * E * H], mm_dt, kind="ExternalInput").ap()
    w2_d = nc.dram_tensor(
        "w2sel", [BPC, P, E * 4 * D], y_dt, kind="ExternalInput"
    ).ap()
    if has_b2:
        b2_d = nc.dram_tensor(
            "b2sel", [BPC, 1, E * D], y_dt, kind="ExternalInput"
        ).ap()
    if dedup:
        flag_d = nc.dram_tensor(
            "w2flag", [1, BPC], mybir.dt.int32, kind="ExternalInput"
        ).ap()
    out_d = nc.dram_tensor("out", [BPC, P, 2 * D], o_dt, kind="ExternalOutput").ap()

    with tile.TileContext(nc) as tc:
        with (
            tc.tile_pool(name="const", bufs=1) as const,
            tc.tile_pool(name="io", bufs=io_bufs) as io,
            tc.tile_pool(name="mid", bufs=mid_bufs) as mid,
            tc.tile_pool(name="psum", bufs=1, space="PSUM") as psp,
        ):
            phi_sb = const.tile([P, 2, ES], mm_dt)
            if split_start:
                phi_v = phi_d.rearrange("p (c s) -> p c s", c=2)
                for dc in range(2):
                    nc.sync.dma_start(out=phi_sb[:, dc, :], in_=phi_v[:, dc, :])
            else:
                nc.sync.dma_start(out=phi_sb, in_=phi_d)
            w1_sb = const.tile([P, 2, E, H], mm_dt)
            if not w1_late:
                nc.sync.dma_start(out=w1_sb, in_=w1_d)
            if dedup:
                # batches are host-sorted by action; w2sel lives in TWO
                # alternating persistent tiles (parity ib%2) and is re-loaded
                # only when the action differs from two batches back
                # (runtime-conditional DMA, flags from the w2flag input).
                w2_fix0 = const.tile([P, E, 4, D], y_dt)
                w2_fix1 = const.tile([P, E, 4, D], y_dt)
                w2_fix = [w2_fix0, w2_fix1]
                flags_sb = const.tile([1, BPC], mybir.dt.int32)
                nc.sync.dma_start(out=flags_sb, in_=flag_d)

            def stage1(ib):
                if merge_oo:
                    # obs and obsT ride one DMA; host stores them adjacently
                    oo_sb = io.tile([P, 4, D], mm_dt, tag="oo")
                    nc.sync.dma_start(
                        out=oo_sb, in_=oo_d[ib].rearrange("p (c d) -> p c d", c=4)
                    )
                    obsT_sb = oo_sb[:, 2:4, :]
                    obs_sb = oo_sb[:, 0:2, :]
                else:
                    obsT_sb = io.tile([P, 2, M], mm_dt, tag="obsT")
                    nc.sync.dma_start(out=obsT_sb, in_=obsT_d[ib])
                    obs_sb = io.tile([P, 2, D], mm_dt, tag="obs")
                    nc.sync.dma_start(out=obs_sb, in_=obs_d[ib])
                if ib == 0 and w1_late:
                    # logits only need phi+obsT, so deferring the w1 const
                    # load lets PE start ~3us earlier.
                    nc.sync.dma_start(out=w1_sb, in_=w1_d)
                w2_src = w2_d[ib].rearrange("p (e k) -> p e k", e=E)
                if dedup:
                    w2_sb = w2_fix[ib % 2]
                    if ib < 2:
                        for e in range(E):
                            nc.sync.dma_start(out=w2_sb[:, e], in_=w2_src[:, e])
                    else:
                        cv = nc.sync.value_load(
                            flags_sb[0:1, ib : ib + 1], min_val=0, max_val=1
                        )
                        for e in range(E):
                            nc.sync.dma_start(
                                out=w2_sb[:, e], in_=w2_src[:, e],
                                cond=cv, cond_hint=False,
                            )
                else:
                    w2_sb = io.tile([P, E, 4, D], y_dt, tag="w2")
                    if w2_one:
                        nc.sync.dma_start(out=w2_sb, in_=w2_src)
                    else:
                        for e in range(E):
                            nc.sync.dma_start(out=w2_sb[:, e], in_=w2_src[:, e])
                if has_b2:
                    # broadcast b2sel[e] across the 64 slot partitions of
                    # each expert: two 0-stride partition DMAs (pg = e % 2)
                    b2_bc = io.tile([P, 2, D], mm_dt, tag="b2")
                    for pg in range(2):
                        srcap = bass.AP(
                            tensor=b2_d.tensor,
                            offset=ib * E * D + pg * D,
                            ap=[[0, S], [2 * D, 2], [1, D]],
                        )
                        nc.sync.dma_start(
                            out=b2_bc[pg * S : (pg + 1) * S, :, :], in_=srcap
                        )

                # logits [m, es] (2 m-chunks), contracting d (2 chunks)
                lg_ps = psp.tile([P, 2, ES], F32, tag="lg", bufs=lg_bufs)
                for mc in range(2):
                    for dc in range(2):
                        nc.tensor.matmul(
                            lg_ps[:, mc, :],
                            obsT_sb[:, dc, ts(mc, P)],
                            phi_sb[:, dc, :],
                            start=(dc == 0),
                            stop=(dc == 1),
                        )
                exp_l = mid.tile([P, 2, ES], mm_dt, tag="expl")
                rsum = mid.tile([P, 2], F32, tag="rsum")
                for mc in range(2):
                    nc.scalar.activation(
                        exp_l[:, mc, :], lg_ps[:, mc, :], AF.Exp,
                        accum_out=rsum[:, mc : mc + 1],
                    )

                # logitsT [es, m] (2 es-chunks)
                lgT_ps = psp.tile([P, 2, M], F32, tag="lg" if share_lg else "lgT", bufs=lg_bufs if share_lg else 1)
                for ec in range(2):
                    for dc in range(2):
                        nc.tensor.matmul(
                            lgT_ps[:, ec, :],
                            phi_sb[:, dc, ts(ec, P)],
                            obsT_sb[:, dc, :],
                            start=(dc == 0),
                            stop=(dc == 1),
                        )
                exp_lT = mid.tile([P, 2, M], mm_dt, tag="explT")
                csum = mid.tile([P, 2], F32, tag="csum")
                for ec in range(2):
                    nc.scalar.activation(
                        exp_lT[:, ec, :], lgT_ps[:, ec, :], AF.Exp,
                        accum_out=csum[:, ec : ec + 1],
                    )

                recip_c = mid.tile([P, 2], F32, tag="rc")
                nc.vector.reciprocal(recip_c, rsum)
                recip_d = mid.tile([P, 2], F32, tag="rd")
                nc.vector.reciprocal(recip_d, csum)

                # slotsT [d, es] = obs.T @ exp_l (unnormalized dispatch)
                sl_ps = psp.tile([P, 2, ES], F32, tag="sl")
                for dc in range(2):
                    for mc in range(2):
                        nc.tensor.matmul(
                            sl_ps[:, dc, :],
                            obs_sb[:, mc, ts(dc, P)],
                            exp_l[:, mc, :],
                            start=(mc == 0),
                            stop=(mc == 1),
                        )
                slots_sb = mid.tile([P, 2, ES], mm_dt, tag="slots")
                nc.vector.tensor_copy(slots_sb, sl_ps)

                return (slots_sb, exp_lT, recip_c, recip_d, w2_sb,
                        b2_bc if has_b2 else None)

            def tail(ib, ctx):
                slots_sb, exp_lT, recip_c, recip_d, w2_sb, b2_bc = ctx
                # pre_h [h, (e,s)] per h-chunk; 4 h-chunks x 4 experts x 2 dc
                # (two half-tiles so ReLU on half 0 overlaps matmuls of half 1)
                h_sb = mid.tile([P, 4, ES], y_dt, tag="h")
                for half in range(2):
                    ph_ps = psp.tile([P, 2, ES], F32, tag="ph", bufs=2)
                    for hc2 in range(2):
                        hc = half * 2 + hc2
                        for e in range(E):
                            for dc in range(2):
                                nc.tensor.matmul(
                                    ph_ps[:, hc2, ds(e * S, S)],
                                    w1_sb[:, dc, e, ts(hc, P)],
                                    slots_sb[:, dc, ds(e * S, S)],
                                    start=(dc == 0),
                                    stop=(dc == 1),
                                )
                    nc.vector.tensor_scalar_max(
                        h_sb[:, half * 2 : half * 2 + 2, :], ph_ps, 0.0
                    )

                # y [es, d]: expert e -> es-chunk e//2, partition off (e%2)*64
                # Dispatch normalizer applied on the PSUM->SBUF copy; the four
                # copies alternate DVE/ACT to balance engine load.
                y_sb = mid.tile([P, 2, D], mm_dt, tag="ysb")
                for e in range(E):
                    ec, po = e // 2, (e % 2) * S
                    y_ps = psp.tile([S, D], F32, tag="y", bufs=y_bufs)
                    for hc in range(4):
                        nc.tensor.matmul(
                            y_ps,
                            h_sb[:, hc, ds(e * S, S)],
                            w2_sb[:, e, hc, :],
                            start=(hc == 0),
                            stop=(hc == 3),
                        )
                    use_dve = ysc == "dve" or (ysc == "alt" and e % 2 == 0)
                    if use_dve:
                        nc.vector.tensor_scalar_mul(
                            y_sb[po : po + S, ec, :], in0=y_ps,
                            scalar1=recip_d[po : po + S, ec : ec + 1],
                        )
                    else:
                        nc.scalar.activation(
                            y_sb[po : po + S, ec, :], y_ps, AF.Copy,
                            scale=recip_d[po : po + S, ec : ec + 1],
                        )
                    if has_b2:
                        nc.vector.tensor_add(
                            y_sb[po : po + S, ec, :],
                            y_sb[po : po + S, ec, :],
                            b2_bc[po : po + S, ec, :],
                        )

                # out [m, d] = exp_lT.T @ y, then combine normalization
                ou_ps = psp.tile([P, 2, D], F32, tag="ou", bufs=ou_bufs)
                for mc in range(2):
                    for ec in range(2):
                        nc.tensor.matmul(
                            ou_ps[:, mc, :],
                            exp_lT[:, ec, ts(mc, P)],
                            y_sb[:, ec, :],
                            start=(ec == 0),
                            stop=(ec == 1),
                        )
                out_sb = io.tile([P, 2, D], o_dt, tag="out")
                for mc in range(2):
                    nc.vector.tensor_scalar_mul(
                        out_sb[:, mc, :], in0=ou_ps[:, mc, :],
                        scalar1=recip_c[:, mc : mc + 1],
                    )
                nc.sync.dma_start(out=out_d[ib], in_=out_sb)

            if PIPELINED_EMIT:
                prev = None
                for ib in range(BPC):
                    ctx = stage1(ib)
                    if prev is not None:
                        tail(ib - 1, prev)
                    prev = ctx
                tail(BPC - 1, prev)
            else:
                for ib in range(BPC):
                    tail(ib, stage1(ib))

    nc.compile()
    return nc


class _Runner:
    """Compile once per process; re-execute via a cached jitted shard_map."""

    def __init__(self, mm_dt=F32, y_dt=None, has_b2=False):
        # The Tile PSUM slot allocator is heuristic and can spuriously fail
        # near capacity; retry a few times.
        last = None
        for _ in range(4):
            try:
                self.nc = build_nc(
                    mm_dt=mm_dt, y_dt=y_dt, has_b2=has_b2, dedup=DEDUP
                )
                break
            except ValueError as e:
                last = e
        else:
            raise last
        self.has_b2 = has_b2
        self._fn = None

    def _build_fn(self):
        import jax
        from jax.sharding import Mesh, PartitionSpec
        from jax.experimental.shard_map import shard_map
        from concourse import bass2jax
        from concourse.bass2jax import _bass_exec_p, partition_id_tensor

        bass2jax.install_neuronx_cc_hook()
        nc = self.nc
        partition_name = (
            nc.partition_id_tensor.name if nc.partition_id_tensor else None
        )
        in_names, out_names, out_avals, zero_outs = [], [], [], []
        for alloc in nc.m.functions[0].allocations:
            if not isinstance(alloc, mybir.MemoryLocationSet):
                continue
            name = alloc.memorylocations[0].name
            if alloc.kind == "ExternalInput":
                if name != partition_name:
                    in_names.append(name)
            elif alloc.kind == "ExternalOutput":
                shape = tuple(alloc.tensor_shape)
                dtype = mybir.dt.np(alloc.dtype)
                out_names.append(name)
                out_avals.append(jax.core.ShapedArray(shape, dtype))
                zero_outs.append(np.zeros(shape, dtype))
        n_params = len(in_names)
        all_in_names = list(in_names) + list(out_names)
        if partition_name is not None:
            all_in_names.append(partition_name)

        def _body(*args):
            operands = list(args)
            if partition_name is not None:
                operands.append(partition_id_tensor())
            outs = _bass_exec_p.bind(
                *operands,
                out_avals=tuple(out_avals),
                in_names=tuple(all_in_names),
                out_names=tuple(out_names),
                lowering_input_output_aliases=(),
                sim_require_finite=True,
                sim_require_nnan=True,
                nc=nc,
            )
            return tuple(outs)

        devices = jax.devices()[:N_CORES]
        assert len(devices) >= N_CORES, (
            f"need {N_CORES} NeuronCores, found {len(jax.devices())}"
        )
        mesh = Mesh(np.asarray(devices), ("core",))
        n_outs = len(out_names)
        sharded = jax.jit(
            shard_map(
                _body,
                mesh=mesh,
                in_specs=(PartitionSpec("core"),) * (n_params + n_outs),
                out_specs=(PartitionSpec("core"),) * n_outs,
                check_rep=False,
            ),
            donate_argnums=tuple(range(n_params, n_params + n_outs)),
            keep_unused=True,
        )
        self._in_names = in_names
        self._out_names = out_names
        self._out_avals = out_avals
        self._zero_outs = zero_outs
        self._fn = sharded

    def run(self, in_maps):
        """in_maps: list of N_CORES dicts name->np.ndarray. Returns per-core
        dict of outputs."""
        if self._fn is None:
            self._build_fn()
        concat_in = [
            np.concatenate([in_maps[c][nm] for c in range(N_CORES)], axis=0)
            for nm in self._in_names
        ]
        concat_zeros = [
            np.zeros((N_CORES * z.shape[0], *z.shape[1:]), z.dtype)
            for z in self._zero_outs
        ]
        out_arrs = self._fn(*concat_in, *concat_zeros)
        return [
            {
                nm: np.asarray(out_arrs[i]).reshape(
                    N_CORES, *self._out_avals[i].shape
                )[c]
                for i, nm in enumerate(self._out_names)
            }
            for c in range(N_CORES)
        ]


_runner_cache = {}


DEDUP = os.environ.get("MOE_DEDUP", "0") == "1"


def _prep_inputs(obs, action, phi, w1, b1, w2, b2):
    obs = np.ascontiguousarray(np.asarray(obs, dtype=np.float32))
    action = np.asarray(action).astype(np.int64)
    phi = np.asarray(phi, dtype=np.float32)
    w1 = np.ascontiguousarray(np.asarray(w1, dtype=np.float32))
    b1 = np.asarray(b1, dtype=np.float32)
    w2 = np.asarray(w2, dtype=np.float32)
    b2 = np.asarray(b2, dtype=np.float32)
    if np.any(b1):
        # The device kernel folds the dispatch-softmax normalizer past the
        # ReLU, which requires b1 == 0 (true for this problem's inputs).
        # Any other input falls back to an exact host computation.
        return None
    # Sort batches by action so equal-action batches are adjacent; the
    # kernel then skips re-loading w2sel when the action repeats. The
    # output rows are un-permuted at the end of kernel().
    if DEDUP:
        order = np.argsort(action, kind="stable")
    else:
        order = np.arange(B)
    obs = obs[order]
    action_s = action[order]
    flags = np.ones(B, np.int32)
    for b in range(B):
        if b % BPC >= 2 and action_s[b] == action_s[b - 2]:
            flags[b] = 0
    obsT = obs.transpose(0, 2, 1)
    # Pre-rearrange everything into the kernel's SBUF layouts (partition dim
    # first, contiguous free) so on-device DMAs are plain [128, N] copies.
    # obs [B,M,D] -> (b, p, mc, d): m = mc*128 + p
    obs_k = np.ascontiguousarray(
        obs.reshape(B, 2, P, D).transpose(0, 2, 1, 3)
    ).reshape(B, P, 2 * D)
    # obsT [B,D,M] -> (b, p, dc, m): d = dc*128 + p
    obsT_k = np.ascontiguousarray(
        obsT.reshape(B, 2, P, M).transpose(0, 2, 1, 3)
    ).reshape(B, P, 2 * M)
    # phi [D,ES] -> (p, dc, es)
    phi_k = np.ascontiguousarray(
        phi.reshape(2, P, ES).transpose(1, 0, 2)
    ).reshape(P, 2 * ES)
    # w1 [E,D,H] -> (p, dc, e, h)
    w1_k = np.ascontiguousarray(
        w1.reshape(E, 2, P, H).transpose(2, 1, 0, 3)
    ).reshape(P, 2 * E * H)
    # per-batch action-selected slices: w2sel [B,E,H,D] -> (b, p, e, hc, d)
    w2r = w2.reshape(E, H, A, D)
    w2sel = w2r[:, :, action_s, :].transpose(2, 0, 1, 3)  # [B,E,H,D]
    w2_k = np.ascontiguousarray(
        w2sel.reshape(B, E, 4, P, D).transpose(0, 3, 1, 2, 4)
    ).reshape(B, P, E * 4 * D)
    has_b2 = bool(np.any(b2))
    b2_k = None
    if has_b2:
        b2r = b2.reshape(E, A, D)
        b2_k = np.ascontiguousarray(
            b2r[:, action_s, :].transpose(1, 0, 2)
        ).reshape(B, 1, E * D)

    np_main = mybir.dt.np(MM_DT)
    np_y = mybir.dt.np(Y_DT)
    obs_k = obs_k.astype(np_main)
    obsT_k = obsT_k.astype(np_main)
    phi_k = phi_k.astype(np_main)
    w1_k = w1_k.astype(np_main)
    w2_k = w2_k.astype(np_y)
    if has_b2:
        b2_k = b2_k.astype(np_y)
    in_maps = []
    for c in range(N_CORES):
        sl = slice(c * BPC, (c + 1) * BPC)
        m = {
            "obs": obs_k[sl],
            "obsT": obsT_k[sl],
            "phi": phi_k,
            "w1": w1_k,
            "w2sel": w2_k[sl],
        }
        if DEDUP:
            m["w2flag"] = flags[sl].reshape(1, BPC)
        if has_b2:
            m["b2sel"] = b2_k[sl]
        in_maps.append(m)
    return in_maps, has_b2, order


def get_runner(has_b2, mm_dt=None, y_dt=None):
    if mm_dt is None:
        mm_dt = MM_DT
    if y_dt is None:
        y_dt = Y_DT
    key = (str(mm_dt), str(y_dt), has_b2)
    if key not in _runner_cache:
        _runner_cache[key] = _Runner(mm_dt=mm_dt, y_dt=y_dt, has_b2=has_b2)
    return _runner_cache[key]


def _numpy_reference(obs, action, phi, w1, b1, w2, b2):
    obs = np.asarray(obs, np.float64)
    logits = np.einsum("bmd,des->bmes", obs, np.asarray(phi, np.float64).reshape(D, E, S))
    lmax = logits.max(axis=1, keepdims=True)
    el = np.exp(logits - lmax)
    dispatch = el / el.sum(axis=1, keepdims=True)
    lf = logits.reshape(B, M, E * S)
    ec_ = np.exp(lf - lf.max(axis=-1, keepdims=True))
    combine = (ec_ / ec_.sum(axis=-1, keepdims=True)).reshape(B, M, E, S)
    slots = np.einsum("bmd,bmes->besd", obs, dispatch)
    h = np.maximum(
        np.einsum("besd,edh->besh", slots, np.asarray(w1, np.float64))
        + np.asarray(b1, np.float64)[None, :, None, :], 0
    )
    y = np.einsum("besh,ehk->besk", h, np.asarray(w2, np.float64)) + np.asarray(
        b2, np.float64
    )[None, :, None, :]
    out = np.einsum("bmes,besk->bmk", combine, y)
    out = out.reshape(B, M, A, D).transpose(0, 2, 1, 3)
    oh = np.eye(A)[np.asarray(action).astype(np.int64)]
    return np.einsum("bamd,ba->bmd", out, oh).astype(np.float32)


def kernel(obs, action, phi, w1, b1, w2, b2):
    prep = _prep_inputs(obs, action, phi, w1, b1, w2, b2)
    if prep is None:
        return _numpy_reference(obs, action, phi, w1, b1, w2, b2)
    in_maps, has_b2, order = prep
    runner = get_runner(has_b2)
    results = None
    last_err = None
    for attempt in range(3):
        try:
            results = runner.run(in_maps)
            break
        except Exception as e:  # transient device wedges recover on retry
            last_err = e
            time.sleep(2.0)
    if results is None:
        raise last_err
    out_k = np.concatenate([results[c]["out"] for c in range(N_CORES)], axis=0)
    # (b, p, mc, d) -> [B, M, D] with m = mc*128 + p; undo the action sort
    out_s = out_k.reshape(B, P, 2, D).transpose(0, 2, 1, 3).reshape(B, M, D)
    out = np.empty_like(out_s)
    out[order] = out_s
    return np.ascontiguousarray(out).astype(np.float32)


# revision 38
# speedup vs baseline: 2224.8104x; 1.0373x over previous
"""Soft-MoE discrete-action transition network — Trainium2 Bass kernel.

Problem shapes (hardcoded):
  obs [B=64, M=256, D=256] f32, action [B=64] i64,
  phi [D, E=4, S=64] f32, w1 [E, D, H=512] f32, b1 [E, H] f32 (zeros),
  w2 [E, H, A*D=4608] f32, b2 [E, A*D] f32 (zeros).  Output [B, M, D] f32.

Strategy:
  * Host gathers the action-selected slice of w2/b2 per batch element
    (w2sel[b] = w2[:, :, a_b*D:(a_b+1)*D]) — the one-hot contraction at the
    end of the reference selects exactly one D-wide block per batch, so
    doing the selection first cuts the dominant matmuls by A=18x
    (~86 GFLOP -> ~13 GFLOP).
  * Data-parallel over batch: 8 batch elements per NeuronCore, params
    replicated, no collectives. All layout rearrangement happens on the
    host so every device DMA is a contiguous [128, N] copy.
  * Matmul operands are fp16 (fp32 would run the PE at 1/4 rate and double
    DMA); PSUM accumulation and softmax plumbing stay fp32. Measured
    end-to-end rel-l2 error ~3.3e-4.
  * Per batch, on device (P=128 partition chunks):
      logits  [m,es] = obsT.T @ phi      (lhsT=obsT[d,m], rhs=phi[d,es])
      logitsT [es,m] = phi.T  @ obsT     (lhsT=phi, rhs=obsT — same operands)
      exp both (ScalarE; accum_out yields both softmax denominators free)
      slotsT  [d,es] = obs.T @ exp_l     (unnormalized dispatch)
      pre_h   [h,es] = w1_e.T @ slotsT   per expert; ReLU (dispatch softmax
                        normalizer folded past ReLU — valid since b1 == 0;
                        nonzero b1 falls back to an exact host computation)
      y       [es,d] = h_e.T @ w2sel_e; scale rows by 1/colsum (dispatch);
                        nonzero b2 is added via a broadcast tile afterwards
      out     [m,d]  = exp_lT.T @ y; scale rows by 1/rowsum (combine)
  * Cost model (TimelineSim): ~55.8 us/core; engine busy: DMA 38us,
    PE 37us, DVE 29us, ACT 26us — a balanced "ridge" kernel.
"""

import os
import sys
import time

import numpy as np

for _p in ("/opt/trn_rl_repo",):
    if os.path.isdir(_p) and _p not in sys.path:
        sys.path.append(_p)

import concourse.bass as bass
import concourse.mybir as mybir
import concourse.tile as tile
from concourse import bacc
from concourse.bass import ds, ts

B, M, D, A = 64, 256, 256, 18
E, S, H = 4, 64, 512
ES = E * S
N_CORES = 8
BPC = B // N_CORES  # batches per core
P = 128
F32 = mybir.dt.float32

AF = mybir.ActivationFunctionType

# Matmul operand dtypes. float32r reinterprets fp32 operands for the PE's
# fast path (1 cycle/row at n>=256 vs 4 for plain fp32). dt_y controls the
# h @ w2sel stage (w2sel dominates DMA traffic; fp16 halves it).
MM_DT = getattr(mybir.dt, os.environ.get("MOE_MM_DT", "float16"))
Y_DT = getattr(mybir.dt, os.environ.get("MOE_Y_DT", "float16"))


def build_nc(mm_dt=F32, y_dt=None, has_b2=False, *, w1_late=True, ysc="dve",
             io_bufs=3, mid_bufs=3, o_dt=F32, lg_bufs=1, share_lg=False,
             y_bufs=2, ou_bufs=1, split_start=False, PIPELINED_EMIT=False,
             dedup=True, merge_oo=False, w2_one=False, w2_bufs=3, w1_split=False,
             out_eng="scalar"):
    """Build the per-core Bass program (one NeuronCore, BPC batches)."""
    if y_dt is None:
        y_dt = mm_dt
    nc = bacc.Bacc("TRN2", target_bir_lowering=False, debug=False)

    # All tensors are pre-rearranged on the host into the exact SBUF layouts,
    # so every DMA is a contiguous [128, N] copy.
    if merge_oo:
        oo_d = nc.dram_tensor(
            "oo", [BPC, P, 2 * D + 2 * M], mm_dt, kind="ExternalInput"
        ).ap()
    else:
        obs_d = nc.dram_tensor(
            "obs", [BPC, P, 2 * D], mm_dt, kind="ExternalInput"
        ).ap()
        obsT_d = nc.dram_tensor(
            "obsT", [BPC, P, 2 * M], mm_dt, kind="ExternalInput"
        ).ap()
    phi_d = nc.dram_tensor("phi", [P, 2 * ES], mm_dt, kind="ExternalInput").ap()
    w1_d = nc.dram_tensor("w1", [P, 2 * E * H], mm_dt, kind="ExternalInput").ap()
    w2_d = nc.dram_tensor(
        "w2sel", [BPC, P, E * 4 * D], y_dt, kind="ExternalInput"
    ).ap()
    if has_b2:
        b2_d = nc.dram_tensor(
            "b2sel", [BPC, 1, E * D], y_dt, kind="ExternalInput"
        ).ap()
    if dedup:
        flag_d = nc.dram_tensor(
            "w2flag", [1, BPC], mybir.dt.int32, kind="ExternalInput"
        ).ap()
    out_d = nc.dram_tensor("out", [BPC, P, 2 * D], o_dt, kind="ExternalOutput").ap()

    with tile.TileContext(nc) as tc:
        with (
            tc.tile_pool(name="const", bufs=1) as const,
            tc.tile_pool(name="io", bufs=io_bufs) as io,
            tc.tile_pool(name="mid", bufs=mid_bufs) as mid,
            tc.tile_pool(name="psum", bufs=1, space="PSUM") as psp,
        ):
            phi_sb = const.tile([P, 2, ES], mm_dt)
            if split_start:
                phi_v = phi_d.rearrange("p (c s) -> p c s", c=2)
                for dc in range(2):
                    nc.sync.dma_start(out=phi_sb[:, dc, :], in_=phi_v[:, dc, :])
            else:
                nc.sync.dma_start(out=phi_sb, in_=phi_d)
            w1_sb = const.tile([P, 2, E, H], mm_dt)
            if not w1_late:
                nc.sync.dma_start(out=w1_sb, in_=w1_d)
            if dedup:
                # batches are host-sorted by action; w2sel lives in TWO
                # alternating persistent tiles (parity ib%2) and is re-loaded
                # only when the action differs from two batches back
                # (runtime-conditional DMA, flags from the w2flag input).
                w2_fix0 = const.tile([P, E, 4, D], y_dt)
                w2_fix1 = const.tile([P, E, 4, D], y_dt)
                w2_fix = [w2_fix0, w2_fix1]
                flags_sb = const.tile([1, BPC], mybir.dt.int32)
                nc.sync.dma_start(out=flags_sb, in_=flag_d)

            def stage1(ib):
                if merge_oo:
                    # obs and obsT ride one DMA; host stores them adjacently
                    oo_sb = io.tile([P, 4, D], mm_dt, tag="oo")
                    nc.sync.dma_start(
                        out=oo_sb, in_=oo_d[ib].rearrange("p (c d) -> p c d", c=4)
                    )
                    obsT_sb = oo_sb[:, 2:4, :]
                    obs_sb = oo_sb[:, 0:2, :]
                else:
                    obsT_sb = io.tile([P, 2, M], mm_dt, tag="obsT")
                    nc.sync.dma_start(out=obsT_sb, in_=obsT_d[ib])
                    obs_sb = io.tile([P, 2, D], mm_dt, tag="obs")
                    nc.sync.dma_start(out=obs_sb, in_=obs_d[ib])
                if ib == 0 and w1_late:
                    # logits only need phi+obsT, so deferring the w1 const
                    # load lets PE start ~3us earlier.
                    if w1_split:
                        w1_v = w1_d.rearrange("p (c k) -> p c k", c=2)
                        for dc in range(2):
                            nc.sync.dma_start(
                                out=w1_sb[:, dc], in_=w1_v[:, dc]
                            )
                    else:
                        nc.sync.dma_start(out=w1_sb, in_=w1_d)
                w2_src = w2_d[ib].rearrange("p (e k) -> p e k", e=E)
                if dedup:
                    w2_sb = w2_fix[ib % 2]
                    if ib < 2:
                        for e in range(E):
                            nc.sync.dma_start(out=w2_sb[:, e], in_=w2_src[:, e])
                    else:
                        cv = nc.sync.value_load(
                            flags_sb[0:1, ib : ib + 1], min_val=0, max_val=1
                        )
                        for e in range(E):
                            nc.sync.dma_start(
                                out=w2_sb[:, e], in_=w2_src[:, e],
                                cond=cv, cond_hint=False,
                            )
                else:
                    w2_sb = io.tile([P, E, 4, D], y_dt, tag="w2", bufs=w2_bufs)
                    if w2_one:
                        nc.sync.dma_start(out=w2_sb, in_=w2_src)
                    else:
                        for e in range(E):
                            nc.sync.dma_start(out=w2_sb[:, e], in_=w2_src[:, e])
                if has_b2:
                    # broadcast b2sel[e] across the 64 slot partitions of
                    # each expert: two 0-stride partition DMAs (pg = e % 2)
                    b2_bc = io.tile([P, 2, D], mm_dt, tag="b2")
                    for pg in range(2):
                        srcap = bass.AP(
                            tensor=b2_d.tensor,
                            offset=ib * E * D + pg * D,
                            ap=[[0, S], [2 * D, 2], [1, D]],
                        )
                        nc.sync.dma_start(
                            out=b2_bc[pg * S : (pg + 1) * S, :, :], in_=srcap
                        )

                # logits [m, es] (2 m-chunks), contracting d (2 chunks)
                lg_ps = psp.tile([P, 2, ES], F32, tag="lg", bufs=lg_bufs)
                for mc in range(2):
                    for dc in range(2):
                        nc.tensor.matmul(
                            lg_ps[:, mc, :],
                            obsT_sb[:, dc, ts(mc, P)],
                            phi_sb[:, dc, :],
                            start=(dc == 0),
                            stop=(dc == 1),
                        )
                exp_l = mid.tile([P, 2, ES], mm_dt, tag="expl")
                rsum = mid.tile([P, 2], F32, tag="rsum")
                for mc in range(2):
                    nc.scalar.activation(
                        exp_l[:, mc, :], lg_ps[:, mc, :], AF.Exp,
                        accum_out=rsum[:, mc : mc + 1],
                    )

                # logitsT [es, m] (2 es-chunks)
                lgT_ps = psp.tile([P, 2, M], F32, tag="lg" if share_lg else "lgT", bufs=lg_bufs if share_lg else 1)
                for ec in range(2):
                    for dc in range(2):
                        nc.tensor.matmul(
                            lgT_ps[:, ec, :],
                            phi_sb[:, dc, ts(ec, P)],
                            obsT_sb[:, dc, :],
                            start=(dc == 0),
                            stop=(dc == 1),
                        )
                exp_lT = mid.tile([P, 2, M], mm_dt, tag="explT")
                csum = mid.tile([P, 2], F32, tag="csum")
                for ec in range(2):
                    nc.scalar.activation(
                        exp_lT[:, ec, :], lgT_ps[:, ec, :], AF.Exp,
                        accum_out=csum[:, ec : ec + 1],
                    )

                recip_c = mid.tile([P, 2], F32, tag="rc")
                nc.vector.reciprocal(recip_c, rsum)
                recip_d = mid.tile([P, 2], F32, tag="rd")
                nc.vector.reciprocal(recip_d, csum)

                # slotsT [d, es] = obs.T @ exp_l (unnormalized dispatch)
                sl_ps = psp.tile([P, 2, ES], F32, tag="sl")
                for dc in range(2):
                    for mc in range(2):
                        nc.tensor.matmul(
                            sl_ps[:, dc, :],
                            obs_sb[:, mc, ts(dc, P)],
                            exp_l[:, mc, :],
                            start=(mc == 0),
                            stop=(mc == 1),
                        )
                slots_sb = mid.tile([P, 2, ES], mm_dt, tag="slots")
                nc.vector.tensor_copy(slots_sb, sl_ps)

                return (slots_sb, exp_lT, recip_c, recip_d, w2_sb,
                        b2_bc if has_b2 else None)

            def tail(ib, ctx):
                slots_sb, exp_lT, recip_c, recip_d, w2_sb, b2_bc = ctx
                # pre_h [h, (e,s)] per h-chunk; 4 h-chunks x 4 experts x 2 dc
                # (two half-tiles so ReLU on half 0 overlaps matmuls of half 1)
                h_sb = mid.tile([P, 4, ES], y_dt, tag="h")
                for half in range(2):
                    ph_ps = psp.tile([P, 2, ES], F32, tag="ph", bufs=2)
                    for hc2 in range(2):
                        hc = half * 2 + hc2
                        for e in range(E):
                            for dc in range(2):
                                nc.tensor.matmul(
                                    ph_ps[:, hc2, ds(e * S, S)],
                                    w1_sb[:, dc, e, ts(hc, P)],
                                    slots_sb[:, dc, ds(e * S, S)],
                                    start=(dc == 0),
                                    stop=(dc == 1),
                                )
                    nc.vector.tensor_scalar_max(
                        h_sb[:, half * 2 : half * 2 + 2, :], ph_ps, 0.0
                    )

                # y [es, d]: expert e -> es-chunk e//2, partition off (e%2)*64
                # Dispatch normalizer applied on the PSUM->SBUF copy; the four
                # copies alternate DVE/ACT to balance engine load.
                y_sb = mid.tile([P, 2, D], mm_dt, tag="ysb")
                for e in range(E):
                    ec, po = e // 2, (e % 2) * S
                    y_ps = psp.tile([S, D], F32, tag="y", bufs=y_bufs)
                    for hc in range(4):
                        nc.tensor.matmul(
                            y_ps,
                            h_sb[:, hc, ds(e * S, S)],
                            w2_sb[:, e, hc, :],
                            start=(hc == 0),
                            stop=(hc == 3),
                        )
                    use_dve = ysc == "dve" or (ysc == "alt" and e % 2 == 0)
                    if use_dve:
                        nc.vector.tensor_scalar_mul(
                            y_sb[po : po + S, ec, :], in0=y_ps,
                            scalar1=recip_d[po : po + S, ec : ec + 1],
                        )
                    else:
                        nc.scalar.activation(
                            y_sb[po : po + S, ec, :], y_ps, AF.Copy,
                            scale=recip_d[po : po + S, ec : ec + 1],
                        )
                    if has_b2:
                        nc.vector.tensor_add(
                            y_sb[po : po + S, ec, :],
                            y_sb[po : po + S, ec, :],
                            b2_bc[po : po + S, ec, :],
                        )

                # out [m, d] = exp_lT.T @ y, then combine normalization
                ou_ps = psp.tile([P, 2, D], F32, tag="ou", bufs=ou_bufs)
                for mc in range(2):
                    for ec in range(2):
                        nc.tensor.matmul(
                            ou_ps[:, mc, :],
                            exp_lT[:, ec, ts(mc, P)],
                            y_sb[:, ec, :],
                            start=(ec == 0),
                            stop=(ec == 1),
                        )
                out_sb = io.tile([P, 2, D], o_dt, tag="out")
                for mc in range(2):
                    nc.vector.tensor_scalar_mul(
                        out_sb[:, mc, :], in0=ou_ps[:, mc, :],
                        scalar1=recip_c[:, mc : mc + 1],
                    )
                # Stores ride the ACT HWDGE ring: on SP they would sit in
                # the FIFO ahead of the next batch's weight loads and
                # head-of-line block them. The last store goes back to SP,
                # whose queue is empty by then, to shorten the tail.
                out_q = {"gpsimd": nc.gpsimd, "sync": nc.sync,
                         "scalar": nc.scalar}[out_eng]
                if ib == BPC - 1:
                    # last batch: SP queue is empty; ship each half as soon
                    # as its scale finishes
                    ov = out_d[ib].rearrange("p (c d) -> p c d", c=2)
                    for mc in range(2):
                        nc.sync.dma_start(out=ov[:, mc, :], in_=out_sb[:, mc, :])
                else:
                    out_q.dma_start(out=out_d[ib], in_=out_sb)

            if PIPELINED_EMIT:
                prev = None
                for ib in range(BPC):
                    ctx = stage1(ib)
                    if prev is not None:
                        tail(ib - 1, prev)
                    prev = ctx
                tail(BPC - 1, prev)
            else:
                for ib in range(BPC):
                    tail(ib, stage1(ib))

    nc.compile()
    return nc


class _Runner:
    """Compile once per process; re-execute via a cached jitted shard_map."""

    def __init__(self, mm_dt=F32, y_dt=None, has_b2=False):
        # The Tile PSUM slot allocator is heuristic and can spuriously fail
        # near capacity; retry a few times.
        last = None
        for _ in range(4):
            try:
                self.nc = build_nc(
                    mm_dt=mm_dt, y_dt=y_dt, has_b2=has_b2, dedup=DEDUP
                )
                break
            except ValueError as e:
                last = e
        else:
            raise last
        self.has_b2 = has_b2
        self._fn = None

    def _build_fn(self):
        import jax
        from jax.sharding import Mesh, PartitionSpec
        from jax.experimental.shard_map import shard_map
        from concourse import bass2jax
        from concourse.bass2jax import _bass_exec_p, partition_id_tensor

        bass2jax.install_neuronx_cc_hook()
        nc = self.nc
        partition_name = (
            nc.partition_id_tensor.name if nc.partition_id_tensor else None
        )
        in_names, out_names, out_avals, zero_outs = [], [], [], []
        for alloc in nc.m.functions[0].allocations:
            if not isinstance(alloc, mybir.MemoryLocationSet):
                continue
            name = alloc.memorylocations[0].name
            if alloc.kind == "ExternalInput":
                if name != partition_name:
                    in_names.append(name)
            elif alloc.kind == "ExternalOutput":
                shape = tuple(alloc.tensor_shape)
                dtype = mybir.dt.np(alloc.dtype)
                out_names.append(name)
                out_avals.append(jax.core.ShapedArray(shape, dtype))
                zero_outs.append(np.zeros(shape, dtype))
        n_params = len(in_names)
        all_in_names = list(in_names) + list(out_names)
        if partition_name is not None:
            all_in_names.append(partition_name)

        def _body(*args):
            operands = list(args)
            if partition_name is not None:
                operands.append(partition_id_tensor())
            outs = _bass_exec_p.bind(
                *operands,
                out_avals=tuple(out_avals),
                in_names=tuple(all_in_names),
                out_names=tuple(out_names),
                lowering_input_output_aliases=(),
                sim_require_finite=True,
                sim_require_nnan=True,
                nc=nc,
            )
            return tuple(outs)

        devices = jax.devices()[:N_CORES]
        assert len(devices) >= N_CORES, (
            f"need {N_CORES} NeuronCores, found {len(jax.devices())}"
        )
        mesh = Mesh(np.asarray(devices), ("core",))
        n_outs = len(out_names)
        sharded = jax.jit(
            shard_map(
                _body,
                mesh=mesh,
                in_specs=(PartitionSpec("core"),) * (n_params + n_outs),
                out_specs=(PartitionSpec("core"),) * n_outs,
                check_rep=False,
            ),
            donate_argnums=tuple(range(n_params, n_params + n_outs)),
            keep_unused=True,
        )
        self._in_names = in_names
        self._out_names = out_names
        self._out_avals = out_avals
        self._zero_outs = zero_outs
        self._fn = sharded

    def run(self, in_maps):
        """in_maps: list of N_CORES dicts name->np.ndarray. Returns per-core
        dict of outputs."""
        if self._fn is None:
            self._build_fn()
        concat_in = [
            np.concatenate([in_maps[c][nm] for c in range(N_CORES)], axis=0)
            for nm in self._in_names
        ]
        concat_zeros = [
            np.zeros((N_CORES * z.shape[0], *z.shape[1:]), z.dtype)
            for z in self._zero_outs
        ]
        out_arrs = self._fn(*concat_in, *concat_zeros)
        return [
            {
                nm: np.asarray(out_arrs[i]).reshape(
                    N_CORES, *self._out_avals[i].shape
                )[c]
                for i, nm in enumerate(self._out_names)
            }
            for c in range(N_CORES)
        ]


_runner_cache = {}


DEDUP = os.environ.get("MOE_DEDUP", "0") == "1"


def _prep_inputs(obs, action, phi, w1, b1, w2, b2):
    obs = np.ascontiguousarray(np.asarray(obs, dtype=np.float32))
    action = np.asarray(action).astype(np.int64)
    phi = np.asarray(phi, dtype=np.float32)
    w1 = np.ascontiguousarray(np.asarray(w1, dtype=np.float32))
    b1 = np.asarray(b1, dtype=np.float32)
    w2 = np.asarray(w2, dtype=np.float32)
    b2 = np.asarray(b2, dtype=np.float32)
    if np.any(b1):
        # The device kernel folds the dispatch-softmax normalizer past the
        # ReLU, which requires b1 == 0 (true for this problem's inputs).
        # Any other input falls back to an exact host computation.
        return None
    # Sort batches by action so equal-action batches are adjacent; the
    # kernel then skips re-loading w2sel when the action repeats. The
    # output rows are un-permuted at the end of kernel().
    if DEDUP:
        order = np.argsort(action, kind="stable")
    else:
        order = np.arange(B)
    obs = obs[order]
    action_s = action[order]
    flags = np.ones(B, np.int32)
    for b in range(B):
        if b % BPC >= 2 and action_s[b] == action_s[b - 2]:
            flags[b] = 0
    obsT = obs.transpose(0, 2, 1)
    # Pre-rearrange everything into the kernel's SBUF layouts (partition dim
    # first, contiguous free) so on-device DMAs are plain [128, N] copies.
    # obs [B,M,D] -> (b, p, mc, d): m = mc*128 + p
    obs_k = np.ascontiguousarray(
        obs.reshape(B, 2, P, D).transpose(0, 2, 1, 3)
    ).reshape(B, P, 2 * D)
    # obsT [B,D,M] -> (b, p, dc, m): d = dc*128 + p
    obsT_k = np.ascontiguousarray(
        obsT.reshape(B, 2, P, M).transpose(0, 2, 1, 3)
    ).reshape(B, P, 2 * M)
    # phi [D,ES] -> (p, dc, es)
    phi_k = np.ascontiguousarray(
        phi.reshape(2, P, ES).transpose(1, 0, 2)
    ).reshape(P, 2 * ES)
    # w1 [E,D,H] -> (p, dc, e, h)
    w1_k = np.ascontiguousarray(
        w1.reshape(E, 2, P, H).transpose(2, 1, 0, 3)
    ).reshape(P, 2 * E * H)
    # per-batch action-selected slices: w2sel [B,E,H,D] -> (b, p, e, hc, d)
    w2r = w2.reshape(E, H, A, D)
    w2sel = w2r[:, :, action_s, :].transpose(2, 0, 1, 3)  # [B,E,H,D]
    w2_k = np.ascontiguousarray(
        w2sel.reshape(B, E, 4, P, D).transpose(0, 3, 1, 2, 4)
    ).reshape(B, P, E * 4 * D)
    has_b2 = bool(np.any(b2))
    b2_k = None
    if has_b2:
        b2r = b2.reshape(E, A, D)
        b2_k = np.ascontiguousarray(
            b2r[:, action_s, :].transpose(1, 0, 2)
        ).reshape(B, 1, E * D)

    np_main = mybir.dt.np(MM_DT)
    np_y = mybir.dt.np(Y_DT)
    obs_k = obs_k.astype(np_main)
    obsT_k = obsT_k.astype(np_main)
    phi_k = phi_k.astype(np_main)
    w1_k = w1_k.astype(np_main)
    w2_k = w2_k.astype(np_y)
    if has_b2:
        b2_k = b2_k.astype(np_y)
    in_maps = []
    for c in range(N_CORES):
        sl = slice(c * BPC, (c + 1) * BPC)
        m = {
            "obs": obs_k[sl],
            "obsT": obsT_k[sl],
            "phi": phi_k,
            "w1": w1_k,
            "w2sel": w2_k[sl],
        }
        if DEDUP:
            m["w2flag"] = flags[sl].reshape(1, BPC)
        if has_b2:
            m["b2sel"] = b2_k[sl]
        in_maps.append(m)
    return in_maps, has_b2, order


def get_runner(has_b2, mm_dt=None, y_dt=None):
    if mm_dt is None:
        mm_dt = MM_DT
    if y_dt is None:
        y_dt = Y_DT
    key = (str(mm_dt), str(y_dt), has_b2)
    if key not in _runner_cache:
        _runner_cache[key] = _Runner(mm_dt=mm_dt, y_dt=y_dt, has_b2=has_b2)
    return _runner_cache[key]


def _numpy_reference(obs, action, phi, w1, b1, w2, b2):
    obs = np.asarray(obs, np.float64)
    logits = np.einsum("bmd,des->bmes", obs, np.asarray(phi, np.float64).reshape(D, E, S))
    lmax = logits.max(axis=1, keepdims=True)
    el = np.exp(logits - lmax)
    dispatch = el / el.sum(axis=1, keepdims=True)
    lf = logits.reshape(B, M, E * S)
    ec_ = np.exp(lf - lf.max(axis=-1, keepdims=True))
    combine = (ec_ / ec_.sum(axis=-1, keepdims=True)).reshape(B, M, E, S)
    slots = np.einsum("bmd,bmes->besd", obs, dispatch)
    h = np.maximum(
        np.einsum("besd,edh->besh", slots, np.asarray(w1, np.float64))
        + np.asarray(b1, np.float64)[None, :, None, :], 0
    )
    y = np.einsum("besh,ehk->besk", h, np.asarray(w2, np.float64)) + np.asarray(
        b2, np.float64
    )[None, :, None, :]
    out = np.einsum("bmes,besk->bmk", combine, y)
    out = out.reshape(B, M, A, D).transpose(0, 2, 1, 3)
    oh = np.eye(A)[np.asarray(action).astype(np.int64)]
    return np.einsum("bamd,ba->bmd", out, oh).astype(np.float32)


def kernel(obs, action, phi, w1, b1, w2, b2):
    prep = _prep_inputs(obs, action, phi, w1, b1, w2, b2)
    if prep is None:
        return _numpy_reference(obs, action, phi, w1, b1, w2, b2)
    in_maps, has_b2, order = prep
    runner = get_runner(has_b2)
    results = None
    last_err = None
    for attempt in range(3):
        try:
            results = runner.run(in_maps)
            break
        except Exception as e:  # transient device wedges recover on retry
            last_err = e
            time.sleep(2.0)
    if results is None:
        raise last_err
    out_k = np.concatenate([results[c]["out"] for c in range(N_CORES)], axis=0)
    # (b, p, mc, d) -> [B, M, D] with m = mc*128 + p; undo the action sort
    out_s = out_k.reshape(B, P, 2, D).transpose(0, 2, 1, 3).reshape(B, M, D)
    out = np.empty_like(out_s)
    out[order] = out_s
    return np.ascontiguousarray(out).astype(np.float32)
